# revision 3
# baseline (speedup 1.0000x reference)
"""TLGv4 block-sparse self-attention on 8 trn2 NeuronCores.

Sharding: tensor-parallel over the 8 KV groups (1 group = 4 Q heads + 1 K +
1 V head per core). Each core computes its group's QKV projection columns,
RoPE, block-sparse attention for its 4 Q heads, and a row-sharded partial of
the dense output projection (written f16). Host sums the 8 partials
(+ b_dense) in f32.

v2 structure - one interleaved stream, per 512-token chunk n:
  - QKV matmuls for chunk n (wq stationary, hidden^T moving, 3 PSUM accs),
    with the largest not-yet-done pair's score matmuls woven between k-steps
    so its Exps (ACT) run under the QKV burst and never pace the PE
  - RoPE on q/k via DVE (q pre-scaled by 1/sqrt(D)); v^T -> v[t,d] via xbar
    DMA transpose; V carries 64 all-ones columns so PV PSUM rows 64:127
    hold the softmax denominators already broadcast across partitions
    (reciprocal + 4 muls, no DRAM bounce)
  - remaining pairs of the group inline: per chunk, score MM -> Exp ->
    sparsity memsets / causal tri multiply, PV lagging 3 chunks, and the
    previous pair's dense-partial units drained between chunks as
    always-ready PE filler
"""
import numpy as np
from contextlib import ExitStack

import concourse.bacc as bacc
import concourse.bass as bass
import concourse.mybir as mybir
import concourse.tile as tile
from concourse.bass_utils import run_bass_kernel_spmd

F32 = mybir.dt.float32
F16 = mybir.dt.float16
AF = mybir.ActivationFunctionType

S = 2048
HID = 2048
D = 64
H_KV = 8
NQ = 4                      # q heads per kv group
GCOLS = (NQ + 2) * D        # 384 qkv columns per group
NPAIR = S // 128            # 16 pairs of 64-token blocks
SCALE = 1.0 / 8.0           # 1/sqrt(D)
ROPE_BASE = 10000.0
N_CORES = 8


def _pair_chunks(i):
    """128-token k-chunks feeding query pair i (blocks 2i, 2i+1)."""
    chunks = list(range(max(0, i - 8), i + 1))
    if i >= 12:
        chunks = [3] + chunks
    return chunks


def _build_nc():
    nc = bacc.Bacc()

    ht = nc.declare_dram_parameter("ht", [HID, S], F16, isOutput=False)
    wq = nc.declare_dram_parameter("wq", [128, 16 * GCOLS], F16, isOutput=False)
    bq = nc.declare_dram_parameter("bq", [128, 3], F32, isOutput=False)
    wd = nc.declare_dram_parameter("wd", [128, 2 * HID], F16, isOutput=False)
    cosq = nc.declare_dram_parameter("cosq", [128, S], F16, isOutput=False)
    sinq = nc.declare_dram_parameter("sinq", [128, S], F16, isOutput=False)
    cosk = nc.declare_dram_parameter("cosk", [64, S], F16, isOutput=False)
    sink = nc.declare_dram_parameter("sink", [64, S], F16, isOutput=False)
    tri = nc.declare_dram_parameter("tri", [128, 128], F16, isOutput=False)
    out = nc.declare_dram_parameter("out", [S, HID], F16, isOutput=True)

    with tile.TileContext(nc) as tc, ExitStack() as ctx:
        consts = ctx.enter_context(tc.tile_pool(name="consts", bufs=1))
        persist = ctx.enter_context(tc.tile_pool(name="persist", bufs=1))
        hp = ctx.enter_context(tc.tile_pool(name="hp", bufs=12))
        rp = ctx.enter_context(tc.tile_pool(name="rope", bufs=2))
        att = ctx.enter_context(tc.tile_pool(name="att", bufs=14))
        ob = ctx.enter_context(tc.tile_pool(name="ob", bufs=3))
        small = ctx.enter_context(tc.tile_pool(name="small", bufs=2))
        # PSUM banks: pss 3 + shr (qkv-acc / ctx) 3 + psd 2 = 8
        pss = ctx.enter_context(tc.tile_pool(name="pss", bufs=3, space="PSUM"))
        shr = ctx.enter_context(tc.tile_pool(name="shr", bufs=3, space="PSUM"))
        psd = ctx.enter_context(tc.tile_pool(name="psd", bufs=2, space="PSUM"))

        wq_sb = consts.tile([128, 16 * GCOLS], F16)
        wd_sb = consts.tile([128, 2 * HID], F16)
        bq_sb = consts.tile([128, 3], F32)
        cosq_sb = consts.tile([128, S], F16)
        sinq_sb = consts.tile([128, S], F16)
        cosk_sb = consts.tile([64, S], F16)
        sink_sb = consts.tile([64, S], F16)
        tri_sb = consts.tile([128, 128], F16)
        expb = consts.tile([128, 1], F32)

        # persistent activations
        qkv = [persist.tile([128, S], F16, tag=f"qkv{m}", name=f"qkv{m}")
               for m in range(3)]
        qS = persist.tile([64, NQ * S], F16)      # [d, pair*512 + h*128 + t]
        kT = persist.tile([64, S], F16)           # [d, t]
        v_sb = persist.tile([128, 16 * 128], F16)  # [t, chunk*128 + (d|ones)]
        ctx_sb = persist.tile([128, 2 * S], F16)  # [(h%2)*64+d, (h//2)*2048+t]

        # critical-path-first DMA order: wq chunk 0, first hidden chunks,
        # then small constants
        nc.sync.dma_start(out=wq_sb[:, 0:GCOLS], in_=wq[:, 0:GCOLS])
        hch0 = []
        for kq in range(2):
            hch = hp.tile([128, 1024], F16, tag="hch", name="hch")
            src = ht[kq * 256:(kq + 1) * 256, 0:512].rearrange(
                "(c p) t -> p c t", p=128)
            nc.sync.dma_start(out=hch[:].rearrange(
                "p (c t) -> p c t", c=2), in_=src)
            hch0.append(hch)
        nc.vector.memset(expb[:], -5.0)
        for t_, src_ in ((bq_sb, bq), (tri_sb, tri)):
            nc.sync.dma_start(out=t_[:], in_=src_[:, :])
        v_r = v_sb[:].rearrange("p (c w) -> p c w", w=128)
        nc.vector.memset(v_r[:, :, 64:128], 1.0)

        # ---- dense emission (pair i), split into interleavable units ----
        def dense_units(i):
            ost = ob.tile([128, 2048], F16, tag="ost", name="ost")
            units = []

            def mk(nn):
                def emit():
                    dps = psd.tile([128, 512], F32, tag="d", name="dps")
                    nc.tensor.matmul(dps[:],
                                     ctx_sb[:, i * 128:(i + 1) * 128],
                                     wd_sb[:, nn * 512:(nn + 1) * 512],
                                     start=True, stop=False)
                    nc.tensor.matmul(
                        dps[:],
                        ctx_sb[:, S + i * 128: S + (i + 1) * 128],
                        wd_sb[:, HID + nn * 512: HID + (nn + 1) * 512],
                        start=False, stop=True)
                    if nn % 2 == 0:
                        nc.vector.tensor_copy(ost[:, nn * 512:(nn + 1) * 512],
                                              dps[:])
                    else:
                        nc.scalar.copy(ost[:, nn * 512:(nn + 1) * 512],
                                       dps[:])
                return emit
            for nn in range(4):
                units.append(mk(nn))

            def fin():
                nc.sync.dma_start(out=out[i * 128:(i + 1) * 128, :], in_=ost[:])
            units.append(fin)
            return units

        pend = []  # outstanding dense units of the previous pair

        def drain(k):
            for _ in range(min(k, len(pend))):
                pend.pop(0)()

        # ---- attention helpers ----
        def score_chunk(i, c, exs):
            s_ps = pss.tile([128, 512], F32, tag="s", name="s_ps")
            nc.tensor.matmul(s_ps[:], kT[:, c * 128:(c + 1) * 128],
                             qS[:, i * 512:(i + 1) * 512],
                             start=True, stop=True)
            ex = att.tile([128, 512], F16, tag="ex", name="ex")
            nc.scalar.activation(ex[:], s_ps[:], AF.Exp, bias=expb[:])
            exs[c] = ex
            if c == i:  # diagonal: causal mask per head
                for h in range(NQ):
                    nc.vector.tensor_mul(ex[:, h * 128:(h + 1) * 128],
                                         ex[:, h * 128:(h + 1) * 128],
                                         tri_sb[:])
            elif i >= 8 and c == i - 8:
                # first half-block invisible; second half only visible to
                # the odd query block if it is a vertical block
                nc.vector.memset(ex[0:64, :], 0.0)
                if i % 4 != 3:
                    exr = ex[64:128, :].rearrange("p (hh t) -> p hh t", hh=NQ)
                    nc.vector.memset(exr[:, :, 64:128], 0.0)
            elif i >= 12 and c == 3:
                # vertical block 7 lives in chunk 3; block 6 invisible
                nc.vector.memset(ex[0:64, :], 0.0)

        def finish_pair(i, chunks, exs, done_pv=0):
            """PV tail + softmax normalization for pair i."""
            nch = len(chunks)
            ctx_ps = exs["ctx_ps"]
            for t in range(done_pv, nch):
                c = chunks[t]
                nc.tensor.matmul(ctx_ps[:], v_sb[:, c * 128:(c + 1) * 128],
                                 exs[c][:], start=(t == 0),
                                 stop=(t == nch - 1))
            recb = small.tile([64, 512], F32, tag="recb", name="recb")
            nc.vector.reciprocal(recb[:], ctx_ps[64:128, :])
            for h in range(NQ):
                nc.vector.tensor_mul(
                    ctx_sb[(h % 2) * 64:(h % 2) * 64 + 64,
                           (h // 2) * S + i * 128:(h // 2) * S + (i + 1) * 128],
                    ctx_ps[0:64, h * 128:(h + 1) * 128],
                    recb[:, h * 128:(h + 1) * 128])

        def emit_pair(i):
            """Inline pair: scores with dense drains + lag-3 PV interleave."""
            chunks = _pair_chunks(i)
            nch = len(chunks)
            exs = {}
            exs["ctx_ps"] = shr.tile([128, 512], F32, tag="ps512",
                                     name="ctx_ps")
            ctx_ps = exs["ctx_ps"]
            for t, c in enumerate(chunks):
                score_chunk(i, c, exs)
                if t >= 1:
                    drain(1)
                if t >= 3:
                    cpv = chunks[t - 3]
                    nc.tensor.matmul(ctx_ps[:],
                                     v_sb[:, cpv * 128:(cpv + 1) * 128],
                                     exs[cpv][:], start=(t == 3),
                                     stop=False)
            drain(5)
            finish_pair(i, chunks, exs, done_pv=max(0, nch - 3))

        # ---- main interleaved loop over 512-token chunks ----
        for n in range(4):
            nsl = slice(n * 512, (n + 1) * 512)
            # issue this chunk's hidden-state DMAs up front
            hlist = []
            for kq in range(8):
                if n == 0 and kq < 2:
                    hlist.append(hch0[kq])
                    continue
                hch = hp.tile([128, 1024], F16, tag="hch", name="hch")
                src = ht[kq * 256:(kq + 1) * 256, nsl].rearrange(
                    "(c p) t -> p c t", p=128)
                nc.sync.dma_start(out=hch[:].rearrange(
                    "p (c t) -> p c t", c=2), in_=src)
                hlist.append(hch)
                if n == 0:
                    # stagger the remaining qkv weight chunks + dense weights
                    for kn in range(kq * 2 - 3, kq * 2 - 1):
                        if 1 <= kn < 16:
                            nc.sync.dma_start(
                                out=wq_sb[:, kn * GCOLS:(kn + 1) * GCOLS],
                                in_=wq[:, kn * GCOLS:(kn + 1) * GCOLS])
            if n == 0:
                for kn in range(13, 16):
                    nc.sync.dma_start(
                        out=wq_sb[:, kn * GCOLS:(kn + 1) * GCOLS],
                        in_=wq[:, kn * GCOLS:(kn + 1) * GCOLS])

            # delayed pair from the previous group: its scores go between
            # QKV k-steps (exp runs under the QKV burst)
            dchunks, dexs = (None, None)
            if n >= 1:
                dpair = 4 * n - 1
                dchunks = _pair_chunks(dpair)
                dexs = {}

            acc = [shr.tile([128, 512], F32, tag="ps512", name=f"acc{m}")
                   for m in range(3)]
            for kq in range(8):
                drain(1)
                hch = hlist[kq]
                for kk in range(2):
                    kc = kq * 2 + kk
                    for mc in range(3):
                        nc.tensor.matmul(
                            acc[mc][:],
                            wq_sb[:, kc * GCOLS + mc * 128: kc * GCOLS + (mc + 1) * 128],
                            hch[:, kk * 512:(kk + 1) * 512],
                            start=(kc == 0), stop=(kc == 15))
                if dchunks is not None and kq < len(dchunks):
                    score_chunk(dpair, dchunks[kq], dexs)
            if dchunks is not None:
                for c in dchunks[8:]:
                    score_chunk(dpair, c, dexs)

            for t_, src_ in ((cosq_sb, cosq), (sinq_sb, sinq),
                             (cosk_sb, cosk), (sink_sb, sink)):
                nc.sync.dma_start(out=t_[:, nsl], in_=src_[:, nsl])
            if n == 0:
                # dense weights all in section 0: dense(0) nn units need
                # every wd quarter by the time pair 1 drains them
                for q4 in range(4):
                    nc.sync.dma_start(
                        out=wd_sb[:, q4 * 1024:(q4 + 1) * 1024],
                        in_=wd[:, q4 * 1024:(q4 + 1) * 1024])
            for mc in range(3):
                nc.vector.tensor_scalar_add(
                    qkv[mc][:, nsl], acc[mc][:], bq_sb[:, mc:mc + 1])
            # rope on this token chunk
            for ti in range(2):
                qt = qkv[ti]
                rot = rp.tile([128, 512], F16, tag="rot", name="rot")
                for blk in range(4):
                    src = (blk ^ 1) * 32
                    nc.vector.tensor_copy(rot[blk * 32:(blk + 1) * 32, :],
                                          qt[src:src + 32, nsl])
                tmp = rp.tile([128, 512], F16, tag="tmp", name="tmp")
                nc.vector.tensor_mul(tmp[:], qt[:, nsl], cosq_sb[:, nsl])
                nc.vector.tensor_mul(rot[:], rot[:], sinq_sb[:, nsl])
                for half in range(2):  # head 2*ti + half
                    h = 2 * ti + half
                    dst = qS[:, n * 2048:(n + 1) * 2048].rearrange(
                        "p (pp hh t) -> p pp hh t", hh=NQ, t=128)[:, :, h, :]
                    nc.vector.tensor_add(
                        dst,
                        tmp[half * 64:(half + 1) * 64, :].rearrange(
                            "p (pp t) -> p pp t", t=128),
                        rot[half * 64:(half + 1) * 64, :].rearrange(
                            "p (pp t) -> p pp t", t=128))
            # k rope (qkv[2] rows 0:64), reusing rot/tmp slots
            rotk = rp.tile([128, 512], F16, tag="rot", name="rotk")
            nc.vector.tensor_copy(rotk[0:32, :], qkv[2][32:64, nsl])
            nc.vector.tensor_copy(rotk[32:64, :], qkv[2][0:32, nsl])
            tmpk = rp.tile([128, 512], F16, tag="tmp", name="tmpk")
            nc.vector.tensor_mul(tmpk[0:64, :], qkv[2][0:64, nsl],
                                 cosk_sb[:, nsl])
            nc.vector.tensor_mul(rotk[0:64, :], rotk[0:64, :], sink_sb[:, nsl])
            nc.vector.tensor_add(kT[:, nsl], tmpk[0:64, :], rotk[0:64, :])
            # v^T -> v[t, d] via xbar DMA transpose per 128-token chunk
            for cc in range(4):
                c = 4 * n + cc
                nc.sync.dma_start_transpose(
                    out=v_sb[:, c * 128:c * 128 + 64],
                    in_=qkv[2][64:128, c * 128:(c + 1) * 128])
            # finish the delayed pair, then this group's first three pairs
            if dchunks is not None:
                dexs["ctx_ps"] = shr.tile([128, 512], F32, tag="ps512",
                                          name="ctx_ps")
                drain(5)
                finish_pair(dpair, dchunks, dexs)
                pend.extend(dense_units(dpair))
            for p in range(4 * n, 4 * n + 3):
                emit_pair(p)
                pend.extend(dense_units(p))
        # last pair never got delayed
        emit_pair(15)
        pend.extend(dense_units(15))
        drain(len(pend))

    nc.finalize()
    return nc


_NC_CACHE = {}


def _get_nc():
    if "nc" not in _NC_CACHE:
        _NC_CACHE["nc"] = _build_nc()
    return _NC_CACHE["nc"]


def _host_inputs(hidden_states, w_qkv, b_qkv, w_dense):
    h = np.asarray(hidden_states, dtype=np.float32).reshape(S, HID)
    w_qkv = np.asarray(w_qkv, dtype=np.float32)
    b_qkv = np.asarray(b_qkv, dtype=np.float32)
    w_dense = np.asarray(w_dense, dtype=np.float32)

    ht = np.ascontiguousarray(h.T).astype(np.float16)

    inv = 1.0 / (ROPE_BASE ** (np.arange(0, D, 2, dtype=np.float32) / D))
    ang = np.arange(S, dtype=np.float32)[:, None] * inv[None, :]   # [S, 32]
    cosT = np.ascontiguousarray(np.cos(ang).T.astype(np.float32))  # [32, S]
    sinT = np.ascontiguousarray(np.sin(ang).T.astype(np.float32))
    cosq = (np.tile(cosT, (4, 1)) * SCALE).astype(np.float16)
    sinq = (np.concatenate([-sinT, sinT, -sinT, sinT], 0) * SCALE).astype(np.float16)
    cosk = np.tile(cosT, (2, 1)).astype(np.float16)
    sink = np.concatenate([-sinT, sinT], 0).astype(np.float16)

    tri = np.triu(np.ones((128, 128), np.float16))

    in_maps = []
    for g in range(N_CORES):
        wqg = w_qkv[g * GCOLS:(g + 1) * GCOLS, :].T          # [HID, 384]
        wq_t = np.ascontiguousarray(
            wqg.reshape(16, 128, GCOLS).transpose(1, 0, 2).reshape(128, 16 * GCOLS)).astype(np.float16)
        bqg = np.ascontiguousarray(
            b_qkv[g * GCOLS:(g + 1) * GCOLS].reshape(3, 128).T)
        wdg = w_dense[:, g * NQ * D:(g + 1) * NQ * D].T      # [256, HID]
        wd_t = np.ascontiguousarray(
            wdg.reshape(2, 128, HID).transpose(1, 0, 2).reshape(128, 2 * HID)).astype(np.float16)
        in_maps.append({
            "ht": ht, "wq": wq_t, "bq": bqg, "wd": wd_t,
            "cosq": np.ascontiguousarray(cosq), "sinq": np.ascontiguousarray(sinq),
            "cosk": np.ascontiguousarray(cosk), "sink": np.ascontiguousarray(sink),
            "tri": tri,
        })
    return in_maps


def run_device(hidden_states, w_qkv, b_qkv, w_dense, **run_kwargs):
    nc = _get_nc()
    in_maps = _host_inputs(hidden_states, w_qkv, b_qkv, w_dense)
    return run_bass_kernel_spmd(nc, in_maps, list(range(N_CORES)), **run_kwargs)


def kernel(hidden_states, w_qkv, b_qkv, w_dense, b_dense):
    res = run_device(hidden_states, w_qkv, b_qkv, w_dense)
    acc = np.zeros((S, HID), dtype=np.float32)
    for r in res.results:
        acc += r["out"].astype(np.float32)
    acc += np.asarray(b_dense, dtype=np.float32)[None, :]
    return acc.reshape(1, S, HID)


# revision 16
# speedup vs baseline: 1.0020x; 1.0020x over previous
"""TLGv4 block-sparse self-attention on 8 trn2 NeuronCores.

Sharding: tensor-parallel over the 8 KV groups (1 group = 4 Q heads + 1 K +
1 V head per core). Each core computes its group's QKV projection columns,
RoPE, block-sparse attention for its 4 Q heads, and a row-sharded partial of
the dense output projection (written f16). Host sums the 8 partials
(+ b_dense) in f32.

v2 structure - one interleaved stream, per 512-token chunk n:
  - QKV matmuls for chunk n (wq stationary, hidden^T moving, 3 PSUM accs),
    with the largest not-yet-done pair's score matmuls woven between k-steps
    so its Exps (ACT) run under the QKV burst and never pace the PE
  - RoPE on q/k via DVE (q pre-scaled by 1/sqrt(D)); v^T -> v[t,d] via xbar
    DMA transpose; V carries 64 all-ones columns so PV PSUM rows 64:127
    hold the softmax denominators already broadcast across partitions
    (reciprocal + 4 muls, no DRAM bounce)
  - remaining pairs of the group inline: per chunk, score MM -> Exp ->
    sparsity memsets / causal tri multiply, PV lagging 3 chunks, and the
    previous pair's dense-partial units drained between chunks as
    always-ready PE filler
"""
import numpy as np
from contextlib import ExitStack

import concourse.bacc as bacc
import concourse.bass as bass
import concourse.mybir as mybir
import concourse.tile as tile
from concourse.bass_utils import run_bass_kernel_spmd

F32 = mybir.dt.float32
F16 = mybir.dt.float16
AF = mybir.ActivationFunctionType

S = 2048
HID = 2048
D = 64
H_KV = 8
NQ = 4                      # q heads per kv group
GCOLS = (NQ + 2) * D        # 384 qkv columns per group
NPAIR = S // 128            # 16 pairs of 64-token blocks
SCALE = 1.0 / 8.0           # 1/sqrt(D)
ROPE_BASE = 10000.0
N_CORES = 8


def _pair_chunks(i):
    """128-token k-chunks feeding query pair i (blocks 2i, 2i+1)."""
    chunks = list(range(max(0, i - 8), i + 1))
    if i >= 12:
        chunks = [3] + chunks
    return chunks


def _build_nc():
    nc = bacc.Bacc()

    # ht2: host-swizzled so each (n, kq) hidden chunk is one contiguous
    # [128, 1024] read: col = ((n*8 + kq)*2 + c)*512 + t holds
    # h[n*512 + t, kq*256 + c*128 + p]
    ht2 = nc.declare_dram_parameter("ht2", [128, 16 * S], F16, isOutput=False)
    wq = nc.declare_dram_parameter("wq", [128, 16 * GCOLS], F16, isOutput=False)
    bq = nc.declare_dram_parameter("bq", [128, 3], F32, isOutput=False)
    wd = nc.declare_dram_parameter("wd", [128, 2 * HID], F16, isOutput=False)
    cosq = nc.declare_dram_parameter("cosq", [128, S], F16, isOutput=False)
    sinq = nc.declare_dram_parameter("sinq", [128, S], F16, isOutput=False)
    cosk = nc.declare_dram_parameter("cosk", [64, S], F16, isOutput=False)
    sink = nc.declare_dram_parameter("sink", [64, S], F16, isOutput=False)
    tri4 = nc.declare_dram_parameter("tri4", [128, 512], F16, isOutput=False)
    out = nc.declare_dram_parameter("out", [S, HID], F16, isOutput=True)

    with tile.TileContext(nc) as tc, ExitStack() as ctx:
        consts = ctx.enter_context(tc.tile_pool(name="consts", bufs=1))
        persist = ctx.enter_context(tc.tile_pool(name="persist", bufs=1))
        hp = ctx.enter_context(tc.tile_pool(name="hp", bufs=12))
        rp = ctx.enter_context(tc.tile_pool(name="rope", bufs=2))
        att = ctx.enter_context(tc.tile_pool(name="att", bufs=14))
        ob = ctx.enter_context(tc.tile_pool(name="ob", bufs=3))
        small = ctx.enter_context(tc.tile_pool(name="small", bufs=2))
        # PSUM banks: pss 3 + shr (qkv-acc / ctx) 3 + psd 2 = 8
        pss = ctx.enter_context(tc.tile_pool(name="pss", bufs=3, space="PSUM"))
        shr = ctx.enter_context(tc.tile_pool(name="shr", bufs=3, space="PSUM"))
        psd = ctx.enter_context(tc.tile_pool(name="psd", bufs=2, space="PSUM"))

        wq_sb = consts.tile([128, 16 * GCOLS], F16)
        wd_sb = consts.tile([128, 2 * HID], F16)
        bq_sb = consts.tile([128, 3], F32)
        cosq_sb = consts.tile([128, S], F16)
        sinq_sb = consts.tile([128, S], F16)
        cosk_sb = consts.tile([64, S], F16)
        sink_sb = consts.tile([64, S], F16)
        tri_sb = consts.tile([128, 512], F16)
        expb = consts.tile([128, 1], F32)

        # persistent activations
        qkv = [persist.tile([128, S], F16, tag=f"qkv{m}", name=f"qkv{m}")
               for m in range(3)]
        qS = persist.tile([64, NQ * S], F16)      # [d, pair*512 + h*128 + t]
        kT = persist.tile([64, S], F16)           # [d, t]
        v_sb = persist.tile([128, 16 * 128], F16)  # [t, chunk*128 + (d|ones)]
        ctx_sb = persist.tile([128, 2 * S], F16)  # [(h%2)*64+d, (h//2)*2048+t]

        # critical-path-first DMA order: wq chunk 0, first hidden chunks,
        # then small constants
        nc.sync.dma_start(out=wq_sb[:, 0:GCOLS], in_=wq[:, 0:GCOLS])
        hch0 = []
        for kq in range(2):
            hch = hp.tile([128, 1024], F16, tag="hch", name="hch")
            nc.sync.dma_start(out=hch[:],
                              in_=ht2[:, kq * 1024:(kq + 1) * 1024])
            hch0.append(hch)
        nc.vector.memset(expb[:], -5.0)
        for t_, src_ in ((bq_sb, bq), (tri_sb, tri4)):
            nc.sync.dma_start(out=t_[:], in_=src_[:, :])
        v_r = v_sb[:].rearrange("p (c w) -> p c w", w=128)
        nc.vector.memset(v_r[:, :, 64:128], 1.0)

        # ---- dense emission (pair i), split into interleavable units ----
        def dense_units(i):
            ost = ob.tile([128, 2048], F16, tag="ost", name="ost")
            units = []

            def mk(nn):
                def emit():
                    dps = psd.tile([128, 512], F32, tag="d", name="dps")
                    nc.tensor.matmul(dps[:],
                                     ctx_sb[:, i * 128:(i + 1) * 128],
                                     wd_sb[:, nn * 512:(nn + 1) * 512],
                                     start=True, stop=False)
                    nc.tensor.matmul(
                        dps[:],
                        ctx_sb[:, S + i * 128: S + (i + 1) * 128],
                        wd_sb[:, HID + nn * 512: HID + (nn + 1) * 512],
                        start=False, stop=True)
                    if nn % 2 == 0:
                        nc.vector.tensor_copy(ost[:, nn * 512:(nn + 1) * 512],
                                              dps[:])
                    else:
                        nc.scalar.copy(ost[:, nn * 512:(nn + 1) * 512],
                                       dps[:])
                return emit
            for nn in range(4):
                units.append(mk(nn))

            def fin():
                nc.sync.dma_start(out=out[i * 128:(i + 1) * 128, :], in_=ost[:])
            units.append(fin)
            return units

        pend = []  # outstanding dense units of the previous pair

        def drain(k):
            for _ in range(min(k, len(pend))):
                pend.pop(0)()

        # ---- attention helpers ----
        def score_chunk(i, c, exs):
            # "half" chunks: only k-rows 64:127 can be visible - compute
            # exp and PV on the upper partition half only (K=64 PV)
            half = (i >= 8 and c == i - 8) or (i >= 12 and c == 3)
            s_ps = pss.tile([128, 512], F32, tag="s", name="s_ps")
            nc.tensor.matmul(s_ps[:], kT[:, c * 128:(c + 1) * 128],
                             qS[:, i * 512:(i + 1) * 512],
                             start=True, stop=True)
            ex = att.tile([128, 512], F16, tag="ex", name="ex")
            if half:
                nc.scalar.activation(ex[64:128, :], s_ps[64:128, :],
                                     AF.Exp, bias=expb[64:128, :])
                if i % 4 != 3 and c == i - 8:
                    exr = ex[64:128, :].rearrange("p (hh t) -> p hh t", hh=NQ)
                    nc.vector.memset(exr[:, :, 64:128], 0.0)
            else:
                nc.scalar.activation(ex[:], s_ps[:], AF.Exp, bias=expb[:])
                if c == i:  # diagonal: causal mask, all heads in one mul
                    nc.vector.tensor_mul(ex[:], ex[:], tri_sb[:])
            exs[c] = (ex, half)

        def pv(ctx_ps, c, exs, start, stop):
            ex, half = exs[c]
            if half:
                nc.tensor.matmul(ctx_ps[:],
                                 v_sb[64:128, c * 128:(c + 1) * 128],
                                 ex[64:128, :], start=start, stop=stop)
            else:
                nc.tensor.matmul(ctx_ps[:], v_sb[:, c * 128:(c + 1) * 128],
                                 ex[:], start=start, stop=stop)

        def finish_pair(i, chunks, exs, done_pv=0):
            """PV tail + softmax normalization for pair i."""
            nch = len(chunks)
            ctx_ps = exs["ctx_ps"]
            for t in range(done_pv, nch):
                pv(ctx_ps, chunks[t], exs, t == 0, t == nch - 1)
            # reciprocal via exp(-ln(x)) on ACT: DVE `reciprocal` costs
            # ~6.5 cyc/elem (3.4us here) and the fast custom-DVE op
            # miscomputes on HW inside this kernel; two ACT LUT passes are
            # ~0.6us each and well within tolerance
            den = small.tile([64, 512], F32, tag="den", name="den")
            nc.vector.tensor_copy(den[:], ctx_ps[64:128, :])
            recb = small.tile([64, 512], F32, tag="recb", name="recb")
            nc.scalar.activation(recb[:], den[:], AF.Ln)
            nc.scalar.activation(recb[:], recb[:], AF.Exp, scale=-1.0)
            for h in range(NQ):
                nc.vector.tensor_mul(
                    ctx_sb[(h % 2) * 64:(h % 2) * 64 + 64,
                           (h // 2) * S + i * 128:(h // 2) * S + (i + 1) * 128],
                    ctx_ps[0:64, h * 128:(h + 1) * 128],
                    recb[:, h * 128:(h + 1) * 128])

        def emit_pair(i):
            """Inline pair: scores with dense drains + lag-3 PV interleave."""
            chunks = _pair_chunks(i)
            nch = len(chunks)
            exs = {}
            exs["ctx_ps"] = shr.tile([128, 512], F32, tag="ps512",
                                     name="ctx_ps")
            ctx_ps = exs["ctx_ps"]
            for t, c in enumerate(chunks):
                score_chunk(i, c, exs)
                if t >= 1:
                    drain(1)
                if t >= 3:
                    pv(ctx_ps, chunks[t - 3], exs, t == 3, False)
            drain(5)
            finish_pair(i, chunks, exs, done_pv=max(0, nch - 3))

        # ---- main interleaved loop over 512-token chunks ----
        for n in range(4):
            nsl = slice(n * 512, (n + 1) * 512)
            # issue this chunk's hidden-state DMAs up front
            hlist = []
            for kq in range(8):
                if n == 0 and kq < 2:
                    hlist.append(hch0[kq])
                    continue
                hch = hp.tile([128, 1024], F16, tag="hch", name="hch")
                nc.sync.dma_start(
                    out=hch[:],
                    in_=ht2[:, (n * 8 + kq) * 1024:(n * 8 + kq + 1) * 1024])
                hlist.append(hch)
                if n == 0:
                    # stagger the remaining qkv weight chunks + dense weights
                    for kn in range(kq * 2 - 3, kq * 2 - 1):
                        if 1 <= kn < 16:
                            nc.sync.dma_start(
                                out=wq_sb[:, kn * GCOLS:(kn + 1) * GCOLS],
                                in_=wq[:, kn * GCOLS:(kn + 1) * GCOLS])
            if n == 0:
                for kn in range(13, 16):
                    nc.sync.dma_start(
                        out=wq_sb[:, kn * GCOLS:(kn + 1) * GCOLS],
                        in_=wq[:, kn * GCOLS:(kn + 1) * GCOLS])

            # delayed pair from the previous group: its scores go between
            # QKV k-steps (exp runs under the QKV burst)
            dchunks, dexs = (None, None)
            if n >= 1:
                dpair = 4 * n - 1
                dchunks = _pair_chunks(dpair)
                dexs = {}

            acc = [shr.tile([128, 512], F32, tag="ps512", name=f"acc{m}")
                   for m in range(3)]
            for kq in range(8):
                drain(1)
                hch = hlist[kq]
                for kk in range(2):
                    kc = kq * 2 + kk
                    for mc in range(3):
                        nc.tensor.matmul(
                            acc[mc][:],
                            wq_sb[:, kc * GCOLS + mc * 128: kc * GCOLS + (mc + 1) * 128],
                            hch[:, kk * 512:(kk + 1) * 512],
                            start=(kc == 0), stop=(kc == 15))
                if dchunks is not None and kq < len(dchunks):
                    score_chunk(dpair, dchunks[kq], dexs)
            if dchunks is not None:
                for c in dchunks[8:]:
                    score_chunk(dpair, c, dexs)

            for t_, src_ in ((cosq_sb, cosq), (sinq_sb, sinq),
                             (cosk_sb, cosk), (sink_sb, sink)):
                nc.sync.dma_start(out=t_[:, nsl], in_=src_[:, nsl])
            if n == 0:
                # dense weights all in section 0: dense(0) nn units need
                # every wd quarter by the time pair 1 drains them
                for q4 in range(4):
                    nc.sync.dma_start(
                        out=wd_sb[:, q4 * 1024:(q4 + 1) * 1024],
                        in_=wd[:, q4 * 1024:(q4 + 1) * 1024])
            for mc in (2, 0, 1):
                nc.vector.tensor_scalar_add(
                    qkv[mc][:, nsl], acc[mc][:], bq_sb[:, mc:mc + 1])
                if mc == 2:
                    # v^T -> v[t, d] via xbar DMA transpose, issued as soon
                    # as qkv[2] rows 64:128 exist (PVs of this group's pairs
                    # need v_sb)
                    for cc in range(4):
                        c = 4 * n + cc
                        nc.sync.dma_start_transpose(
                            out=v_sb[:, c * 128:c * 128 + 64],
                            in_=qkv[2][64:128, c * 128:(c + 1) * 128])
            # rope on this token chunk
            for ti in range(2):
                qt = qkv[ti]
                rot = rp.tile([128, 512], F16, tag="rot", name="rot")
                for blk in range(4):
                    src = (blk ^ 1) * 32
                    nc.vector.tensor_copy(rot[blk * 32:(blk + 1) * 32, :],
                                          qt[src:src + 32, nsl])
                tmp = rp.tile([128, 512], F16, tag="tmp", name="tmp")
                nc.vector.tensor_mul(tmp[:], qt[:, nsl], cosq_sb[:, nsl])
                nc.vector.tensor_mul(rot[:], rot[:], sinq_sb[:, nsl])
                for half in range(2):  # head 2*ti + half
                    h = 2 * ti + half
                    dst = qS[:, n * 2048:(n + 1) * 2048].rearrange(
                        "p (pp hh t) -> p pp hh t", hh=NQ, t=128)[:, :, h, :]
                    nc.vector.tensor_add(
                        dst,
                        tmp[half * 64:(half + 1) * 64, :].rearrange(
                            "p (pp t) -> p pp t", t=128),
                        rot[half * 64:(half + 1) * 64, :].rearrange(
                            "p (pp t) -> p pp t", t=128))
            # k rope (qkv[2] rows 0:64), reusing rot/tmp slots
            rotk = rp.tile([128, 512], F16, tag="rot", name="rotk")
            nc.vector.tensor_copy(rotk[0:32, :], qkv[2][32:64, nsl])
            nc.vector.tensor_copy(rotk[32:64, :], qkv[2][0:32, nsl])
            tmpk = rp.tile([128, 512], F16, tag="tmp", name="tmpk")
            nc.vector.tensor_mul(tmpk[0:64, :], qkv[2][0:64, nsl],
                                 cosk_sb[:, nsl])
            nc.vector.tensor_mul(rotk[0:64, :], rotk[0:64, :], sink_sb[:, nsl])
            nc.vector.tensor_add(kT[:, nsl], tmpk[0:64, :], rotk[0:64, :])
            # finish the delayed pair, then this group's first three pairs
            if dchunks is not None:
                dexs["ctx_ps"] = shr.tile([128, 512], F32, tag="ps512",
                                          name="ctx_ps")
                drain(5)
                finish_pair(dpair, dchunks, dexs)
                pend.extend(dense_units(dpair))
            for p in range(4 * n, 4 * n + 3):
                emit_pair(p)
                pend.extend(dense_units(p))
        # last pair never got delayed
        emit_pair(15)
        pend.extend(dense_units(15))
        drain(len(pend))

    nc.finalize()
    return nc


_NC_CACHE = {}


def _get_nc():
    if "nc" not in _NC_CACHE:
        _NC_CACHE["nc"] = _build_nc()
    return _NC_CACHE["nc"]


def _host_inputs(hidden_states, w_qkv, b_qkv, w_dense):
    h = np.asarray(hidden_states, dtype=np.float32).reshape(S, HID)
    w_qkv = np.asarray(w_qkv, dtype=np.float32)
    b_qkv = np.asarray(b_qkv, dtype=np.float32)
    w_dense = np.asarray(w_dense, dtype=np.float32)

    # contiguous per-(n,kq) [128, 1024] chunks: col ((n*8+kq)*2+c)*512+t
    # holds h[n*512+t, kq*256+c*128+p]
    ht2 = np.ascontiguousarray(
        h.reshape(4, 512, 8, 2, 128).transpose(4, 0, 2, 3, 1).reshape(
            128, 16 * S)).astype(np.float16)

    inv = 1.0 / (ROPE_BASE ** (np.arange(0, D, 2, dtype=np.float32) / D))
    ang = np.arange(S, dtype=np.float32)[:, None] * inv[None, :]   # [S, 32]
    cosT = np.ascontiguousarray(np.cos(ang).T.astype(np.float32))  # [32, S]
    sinT = np.ascontiguousarray(np.sin(ang).T.astype(np.float32))
    cosq = (np.tile(cosT, (4, 1)) * SCALE).astype(np.float16)
    sinq = (np.concatenate([-sinT, sinT, -sinT, sinT], 0) * SCALE).astype(np.float16)
    cosk = np.tile(cosT, (2, 1)).astype(np.float16)
    sink = np.concatenate([-sinT, sinT], 0).astype(np.float16)

    tri4 = np.tile(np.triu(np.ones((128, 128), np.float16)), (1, 4))

    in_maps = []
    for g in range(N_CORES):
        wqg = w_qkv[g * GCOLS:(g + 1) * GCOLS, :].T          # [HID, 384]
        wq_t = np.ascontiguousarray(
            wqg.reshape(16, 128, GCOLS).transpose(1, 0, 2).reshape(128, 16 * GCOLS)).astype(np.float16)
        bqg = np.ascontiguousarray(
            b_qkv[g * GCOLS:(g + 1) * GCOLS].reshape(3, 128).T)
        wdg = w_dense[:, g * NQ * D:(g + 1) * NQ * D].T      # [256, HID]
        wd_t = np.ascontiguousarray(
            wdg.reshape(2, 128, HID).transpose(1, 0, 2).reshape(128, 2 * HID)).astype(np.float16)
        in_maps.append({
            "ht2": ht2, "wq": wq_t, "bq": bqg, "wd": wd_t,
            "cosq": np.ascontiguousarray(cosq), "sinq": np.ascontiguousarray(sinq),
            "cosk": np.ascontiguousarray(cosk), "sink": np.ascontiguousarray(sink),
            "tri4": tri4,
        })
    return in_maps


def run_device(hidden_states, w_qkv, b_qkv, w_dense, **run_kwargs):
    nc = _get_nc()
    in_maps = _host_inputs(hidden_states, w_qkv, b_qkv, w_dense)
    return run_bass_kernel_spmd(nc, in_maps, list(range(N_CORES)), **run_kwargs)


def kernel(hidden_states, w_qkv, b_qkv, w_dense, b_dense):
    res = run_device(hidden_states, w_qkv, b_qkv, w_dense)
    acc = np.zeros((S, HID), dtype=np.float32)
    for r in res.results:
        acc += r["out"].astype(np.float32)
    acc += np.asarray(b_dense, dtype=np.float32)[None, :]
    return acc.reshape(1, S, HID)


# revision 23
# speedup vs baseline: 1.0671x; 1.0650x over previous
"""TLGv4 block-sparse self-attention on 8 trn2 NeuronCores.

Sharding: tensor-parallel over the 8 KV groups (1 group = 4 Q heads + 1 K +
1 V head per core). Each core computes its group's QKV projection columns,
RoPE, block-sparse attention for its 4 Q heads, and a row-sharded partial of
the dense output projection (written f16). Host sums the 8 partials
(+ b_dense) in f32.

v2 structure - one interleaved stream, per 512-token chunk n:
  - QKV matmuls for chunk n (wq stationary, hidden^T moving, 3 PSUM accs),
    with the largest not-yet-done pair's score matmuls woven between k-steps
    so its Exps (ACT) run under the QKV burst and never pace the PE
  - RoPE on q/k via DVE (q pre-scaled by 1/sqrt(D)); v^T -> v[t,d] via xbar
    DMA transpose; V carries 64 all-ones columns so PV PSUM rows 64:127
    hold the softmax denominators already broadcast across partitions
    (reciprocal + 4 muls, no DRAM bounce)
  - remaining pairs of the group inline: per chunk, score MM -> Exp ->
    sparsity memsets / causal tri multiply, PV lagging 3 chunks, and the
    previous pair's dense-partial units drained between chunks as
    always-ready PE filler
"""
import numpy as np
from contextlib import ExitStack

import concourse.bacc as bacc
import concourse.bass as bass
import concourse.mybir as mybir
import concourse.tile as tile
from concourse.bass_utils import run_bass_kernel_spmd

F32 = mybir.dt.float32
F16 = mybir.dt.float16
AF = mybir.ActivationFunctionType

S = 2048
HID = 2048
D = 64
H_KV = 8
NQ = 4                      # q heads per kv group
GCOLS = (NQ + 2) * D        # 384 qkv columns per group
NPAIR = S // 128            # 16 pairs of 64-token blocks
SCALE = 1.0 / 8.0           # 1/sqrt(D)
ROPE_BASE = 10000.0
N_CORES = 8


def _pair_chunks(i):
    """128-token k-chunks feeding query pair i (blocks 2i, 2i+1)."""
    chunks = list(range(max(0, i - 8), i + 1))
    if i >= 12:
        chunks = [3] + chunks
    return chunks


def _build_nc():
    nc = bacc.Bacc()

    # ht2: host-swizzled so each (n, kq) hidden chunk is one contiguous
    # [128, 1024] read: col = ((n*8 + kq)*2 + c)*512 + t holds
    # h[n*512 + t, kq*256 + c*128 + p]
    ht2 = nc.declare_dram_parameter("ht2", [128, 16 * S], F16, isOutput=False)
    wq = nc.declare_dram_parameter("wq", [128, 16 * GCOLS], F16, isOutput=False)
    bq = nc.declare_dram_parameter("bq", [128, 3], F32, isOutput=False)
    wd = nc.declare_dram_parameter("wd", [128, 2 * HID], F16, isOutput=False)
    cosq = nc.declare_dram_parameter("cosq", [128, S], F16, isOutput=False)
    sinq = nc.declare_dram_parameter("sinq", [128, S], F16, isOutput=False)
    cosk = nc.declare_dram_parameter("cosk", [64, S], F16, isOutput=False)
    sink = nc.declare_dram_parameter("sink", [64, S], F16, isOutput=False)
    tri4 = nc.declare_dram_parameter("tri4", [128, 512], F16, isOutput=False)
    out = nc.declare_dram_parameter("out", [S, HID], F16, isOutput=True)

    scratch = nc.dram_tensor("scratch", [NPAIR, 512], F32)

    with tile.TileContext(nc) as tc, ExitStack() as ctx:
        consts = ctx.enter_context(tc.tile_pool(name="consts", bufs=1))
        persist = ctx.enter_context(tc.tile_pool(name="persist", bufs=1))
        hp = ctx.enter_context(tc.tile_pool(name="hp", bufs=12))
        rp = ctx.enter_context(tc.tile_pool(name="rope", bufs=2))
        att = ctx.enter_context(tc.tile_pool(name="att", bufs=14))
        ob = ctx.enter_context(tc.tile_pool(name="ob", bufs=3))
        small = ctx.enter_context(tc.tile_pool(name="small", bufs=2))
        # PSUM banks: pss 3 + shr (qkv-acc / ctx) 3 + psd 2 = 8
        pss = ctx.enter_context(tc.tile_pool(name="pss", bufs=3, space="PSUM"))
        shr = ctx.enter_context(tc.tile_pool(name="shr", bufs=3, space="PSUM"))
        psd = ctx.enter_context(tc.tile_pool(name="psd", bufs=2, space="PSUM"))

        wq_sb = consts.tile([128, 16 * GCOLS], F16)
        wd_sb = consts.tile([128, 2 * HID], F16)
        bq_sb = consts.tile([128, 3], F32)
        cosq_sb = consts.tile([128, S], F16)
        sinq_sb = consts.tile([128, S], F16)
        cosk_sb = consts.tile([64, S], F16)
        sink_sb = consts.tile([64, S], F16)
        tri_sb = consts.tile([128, 512], F16)
        expb = consts.tile([128, 1], F32)

        # persistent activations
        qkv = [persist.tile([128, S], F16, tag=f"qkv{m}", name=f"qkv{m}")
               for m in range(3)]
        qS = persist.tile([64, NQ * S], F16)      # [d, pair*512 + h*128 + t]
        kT = persist.tile([64, S], F16)           # [d, t]
        v_sb = persist.tile([128, 16 * 128], F16)  # [t, chunk*128 + (d|ones)]
        ctx_sb = persist.tile([128, 2 * S], F16)  # [(h%2)*64+d, (h//2)*2048+t]

        # critical-path-first DMA order: wq chunk 0, first hidden chunks,
        # then small constants
        nc.sync.dma_start(out=wq_sb[:, 0:GCOLS], in_=wq[:, 0:GCOLS])
        hch0 = []
        for kq in range(2):
            hch = hp.tile([128, 1024], F16, tag="hch", name="hch")
            nc.sync.dma_start(out=hch[:],
                              in_=ht2[:, kq * 1024:(kq + 1) * 1024])
            hch0.append(hch)
        nc.vector.memset(expb[:], -5.0)
        for t_, src_ in ((bq_sb, bq), (tri_sb, tri4)):
            nc.sync.dma_start(out=t_[:], in_=src_[:, :])
        v_r = v_sb[:].rearrange("p (c w) -> p c w", w=128)
        nc.vector.memset(v_r[:, :, 64:128], 1.0)

        # ---- dense emission (pair i), split into interleavable units ----
        def dense_units(i):
            ost = ob.tile([128, 2048], F16, tag="ost", name="ost")
            units = []

            def mk(nn):
                def emit():
                    dps = psd.tile([128, 512], F32, tag="d", name="dps")
                    nc.tensor.matmul(dps[:],
                                     ctx_sb[:, i * 128:(i + 1) * 128],
                                     wd_sb[:, nn * 512:(nn + 1) * 512],
                                     start=True, stop=False)
                    nc.tensor.matmul(
                        dps[:],
                        ctx_sb[:, S + i * 128: S + (i + 1) * 128],
                        wd_sb[:, HID + nn * 512: HID + (nn + 1) * 512],
                        start=False, stop=True)
                    if nn % 2 == 0:
                        nc.vector.tensor_copy(ost[:, nn * 512:(nn + 1) * 512],
                                              dps[:])
                    else:
                        nc.scalar.copy(ost[:, nn * 512:(nn + 1) * 512],
                                       dps[:])
                return emit
            for nn in range(4):
                units.append(mk(nn))

            def fin():
                nc.sync.dma_start(out=out[i * 128:(i + 1) * 128, :], in_=ost[:])
            units.append(fin)
            return units

        pend = []   # outstanding dense units
        ready = []  # finished pairs whose dense is not yet queued (lag 1)

        def drain(k):
            for _ in range(min(k, len(pend))):
                pend.pop(0)()

        def pair_done(x):
            # queue pair x's dense one pair later, so its normalize (DMA
            # bounce chain) completes before the dense matmuls drain
            if ready:
                pend.extend(dense_units(ready.pop(0)))
            ready.append(x)

        # ---- attention helpers ----
        def score_chunk(i, c, exs):
            # "half" chunks: only k-rows 64:127 can be visible - compute
            # exp and PV on the upper partition half only (K=64 PV)
            half = (i >= 8 and c == i - 8) or (i >= 12 and c == 3)
            s_ps = pss.tile([128, 512], F32, tag="s", name="s_ps")
            nc.tensor.matmul(s_ps[:], kT[:, c * 128:(c + 1) * 128],
                             qS[:, i * 512:(i + 1) * 512],
                             start=True, stop=True)
            ex = att.tile([128, 512], F16, tag="ex", name="ex")
            if half:
                nc.scalar.activation(ex[64:128, :], s_ps[64:128, :],
                                     AF.Exp, bias=expb[64:128, :])
                if i % 4 != 3 and c == i - 8:
                    exr = ex[64:128, :].rearrange("p (hh t) -> p hh t", hh=NQ)
                    nc.vector.memset(exr[:, :, 64:128], 0.0)
            else:
                nc.scalar.activation(ex[:], s_ps[:], AF.Exp, bias=expb[:])
                if c == i:  # diagonal: causal mask, all heads in one mul
                    nc.vector.tensor_mul(ex[:], ex[:], tri_sb[:])
            exs[c] = (ex, half)

        def pv(ctx_ps, c, exs, start, stop):
            ex, half = exs[c]
            if half:
                nc.tensor.matmul(ctx_ps[:],
                                 v_sb[64:128, c * 128:(c + 1) * 128],
                                 ex[64:128, :], start=start, stop=stop)
            else:
                nc.tensor.matmul(ctx_ps[:], v_sb[:, c * 128:(c + 1) * 128],
                                 ex[:], start=start, stop=stop)

        def finish_pair(i, chunks, exs, done_pv=0):
            """PV tail + softmax normalization for pair i."""
            nch = len(chunks)
            ctx_ps = exs["ctx_ps"]
            for t in range(done_pv, nch):
                pv(ctx_ps, chunks[t], exs, t == 0, t == nch - 1)
            # denominators: DVE `reciprocal` is ~6.5 cyc/elem (3.4us on
            # [64,512]), the fast custom-DVE recip miscomputes on HW here,
            # and ACT Ln/Exp thrashes the activation table. So: [64,8]
            # repack via DMA, 163ns reciprocal, DRAM-bounce partition
            # broadcast (HBM's flat addressing does the cross-partition
            # replication)
            den = small.tile([1, 512], F32, tag="den", name="den")
            nc.scalar.copy(den[:], ctx_ps[64:65, :])
            rec8 = small.tile([64, 8], F32, tag="rec8", name="rec8")
            nc.sync.dma_start(out=rec8[:], in_=den[0:1, :].rearrange(
                "o (p f) -> o p f", p=64))
            nc.vector.reciprocal(rec8[:], rec8[:])
            sc_row = scratch[i:i + 1, :]
            nc.sync.dma_start(out=sc_row.rearrange("o (p f) -> o p f", p=64),
                              in_=rec8[:])
            bcast = small.tile([64, 512], F32, tag="bc", name="bc")
            nc.sync.dma_start(out=bcast[:], in_=bass.AP(
                tensor=sc_row.tensor, offset=sc_row.offset,
                ap=[[0, 64]] + sc_row.ap[1:]))
            for h in range(NQ):
                nc.vector.tensor_mul(
                    ctx_sb[(h % 2) * 64:(h % 2) * 64 + 64,
                           (h // 2) * S + i * 128:(h // 2) * S + (i + 1) * 128],
                    ctx_ps[0:64, h * 128:(h + 1) * 128],
                    bcast[:, h * 128:(h + 1) * 128])

        def emit_pair(i):
            """Inline pair: scores with dense drains + lag-3 PV interleave."""
            chunks = _pair_chunks(i)
            nch = len(chunks)
            exs = {}
            exs["ctx_ps"] = shr.tile([128, 512], F32, tag="ps512",
                                     name="ctx_ps")
            ctx_ps = exs["ctx_ps"]
            for t, c in enumerate(chunks):
                score_chunk(i, c, exs)
                if t >= 2:
                    drain(1)
                if t >= 3:
                    pv(ctx_ps, chunks[t - 3], exs, t == 3, False)
            drain(5)
            finish_pair(i, chunks, exs, done_pv=max(0, nch - 3))

        # ---- main interleaved loop over 512-token chunks ----
        for n in range(4):
            nsl = slice(n * 512, (n + 1) * 512)
            # issue this chunk's hidden-state DMAs up front
            hlist = []
            for kq in range(8):
                if n == 0 and kq < 2:
                    hlist.append(hch0[kq])
                    continue
                hch = hp.tile([128, 1024], F16, tag="hch", name="hch")
                nc.sync.dma_start(
                    out=hch[:],
                    in_=ht2[:, (n * 8 + kq) * 1024:(n * 8 + kq + 1) * 1024])
                hlist.append(hch)
                if n == 0:
                    # stagger the remaining qkv weight chunks + dense weights
                    for kn in range(kq * 2 - 3, kq * 2 - 1):
                        if 1 <= kn < 16:
                            nc.sync.dma_start(
                                out=wq_sb[:, kn * GCOLS:(kn + 1) * GCOLS],
                                in_=wq[:, kn * GCOLS:(kn + 1) * GCOLS])
            if n == 0:
                for kn in range(13, 16):
                    nc.sync.dma_start(
                        out=wq_sb[:, kn * GCOLS:(kn + 1) * GCOLS],
                        in_=wq[:, kn * GCOLS:(kn + 1) * GCOLS])

            # delayed pair from the previous group: its scores go between
            # QKV k-steps (exp runs under the QKV burst); two are
            # front-loaded so the PE has ready work across the section
            # boundary
            dchunks, dexs = (None, None)
            if n >= 1:
                dpair = 4 * n - 1
                dchunks = _pair_chunks(dpair)
                dexs = {}
                for c in dchunks[:2]:
                    score_chunk(dpair, c, dexs)

            acc = [shr.tile([128, 512], F32, tag="ps512", name=f"acc{m}")
                   for m in range(3)]
            for kq in range(8):
                if kq >= 3:
                    drain(1)
                hch = hlist[kq]
                for kk in range(2):
                    kc = kq * 2 + kk
                    for mc in range(3):
                        nc.tensor.matmul(
                            acc[mc][:],
                            wq_sb[:, kc * GCOLS + mc * 128: kc * GCOLS + (mc + 1) * 128],
                            hch[:, kk * 512:(kk + 1) * 512],
                            start=(kc == 0), stop=(kc == 15))
                if dchunks is not None and kq + 2 < len(dchunks):
                    score_chunk(dpair, dchunks[kq + 2], dexs)

            for t_, src_ in ((cosq_sb, cosq), (sinq_sb, sinq),
                             (cosk_sb, cosk), (sink_sb, sink)):
                nc.sync.dma_start(out=t_[:, nsl], in_=src_[:, nsl])
            if n == 0:
                # dense weights all in section 0: dense(0) nn units need
                # every wd quarter by the time pair 1 drains them
                for q4 in range(4):
                    nc.sync.dma_start(
                        out=wd_sb[:, q4 * 1024:(q4 + 1) * 1024],
                        in_=wd[:, q4 * 1024:(q4 + 1) * 1024])
            for mc in (2, 0, 1):
                nc.vector.tensor_scalar_add(
                    qkv[mc][:, nsl], acc[mc][:], bq_sb[:, mc:mc + 1])
                if mc == 2:
                    # v^T -> v[t, d] via xbar DMA transpose, issued as soon
                    # as qkv[2] rows 64:128 exist (PVs of this group's pairs
                    # need v_sb)
                    for cc in range(4):
                        c = 4 * n + cc
                        nc.sync.dma_start_transpose(
                            out=v_sb[:, c * 128:c * 128 + 64],
                            in_=qkv[2][64:128, c * 128:(c + 1) * 128])
            # rope on this token chunk
            for ti in range(2):
                qt = qkv[ti]
                rot = rp.tile([128, 512], F16, tag="rot", name="rot")
                for blk in range(4):
                    src = (blk ^ 1) * 32
                    nc.vector.tensor_copy(rot[blk * 32:(blk + 1) * 32, :],
                                          qt[src:src + 32, nsl])
                tmp = rp.tile([128, 512], F16, tag="tmp", name="tmp")
                nc.vector.tensor_mul(tmp[:], qt[:, nsl], cosq_sb[:, nsl])
                nc.vector.tensor_mul(rot[:], rot[:], sinq_sb[:, nsl])
                for half in range(2):  # head 2*ti + half
                    h = 2 * ti + half
                    dst = qS[:, n * 2048:(n + 1) * 2048].rearrange(
                        "p (pp hh t) -> p pp hh t", hh=NQ, t=128)[:, :, h, :]
                    nc.vector.tensor_add(
                        dst,
                        tmp[half * 64:(half + 1) * 64, :].rearrange(
                            "p (pp t) -> p pp t", t=128),
                        rot[half * 64:(half + 1) * 64, :].rearrange(
                            "p (pp t) -> p pp t", t=128))
            # k rope (qkv[2] rows 0:64), reusing rot/tmp slots
            rotk = rp.tile([128, 512], F16, tag="rot", name="rotk")
            nc.vector.tensor_copy(rotk[0:32, :], qkv[2][32:64, nsl])
            nc.vector.tensor_copy(rotk[32:64, :], qkv[2][0:32, nsl])
            tmpk = rp.tile([128, 512], F16, tag="tmp", name="tmpk")
            nc.vector.tensor_mul(tmpk[0:64, :], qkv[2][0:64, nsl],
                                 cosk_sb[:, nsl])
            nc.vector.tensor_mul(rotk[0:64, :], rotk[0:64, :], sink_sb[:, nsl])
            nc.vector.tensor_add(kT[:, nsl], tmpk[0:64, :], rotk[0:64, :])
            # finish the delayed pair, then this group's first three pairs
            if dchunks is not None:
                dexs["ctx_ps"] = shr.tile([128, 512], F32, tag="ps512",
                                          name="ctx_ps")
                drain(5)
                finish_pair(dpair, dchunks, dexs)
                pair_done(dpair)
            for p in range(4 * n, 4 * n + 3):
                emit_pair(p)
                pair_done(p)
        # last pair never got delayed; pair 14's dense drains inside it
        pend.extend(dense_units(ready.pop(0)))
        emit_pair(15)
        pend.extend(dense_units(15))
        drain(len(pend))

    nc.finalize()
    return nc


_NC_CACHE = {}


def _get_nc():
    if "nc" not in _NC_CACHE:
        _NC_CACHE["nc"] = _build_nc()
    return _NC_CACHE["nc"]


def _host_inputs(hidden_states, w_qkv, b_qkv, w_dense):
    h = np.asarray(hidden_states, dtype=np.float32).reshape(S, HID)
    w_qkv = np.asarray(w_qkv, dtype=np.float32)
    b_qkv = np.asarray(b_qkv, dtype=np.float32)
    w_dense = np.asarray(w_dense, dtype=np.float32)

    # contiguous per-(n,kq) [128, 1024] chunks: col ((n*8+kq)*2+c)*512+t
    # holds h[n*512+t, kq*256+c*128+p]
    ht2 = np.ascontiguousarray(
        h.reshape(4, 512, 8, 2, 128).transpose(4, 0, 2, 3, 1).reshape(
            128, 16 * S)).astype(np.float16)

    inv = 1.0 / (ROPE_BASE ** (np.arange(0, D, 2, dtype=np.float32) / D))
    ang = np.arange(S, dtype=np.float32)[:, None] * inv[None, :]   # [S, 32]
    cosT = np.ascontiguousarray(np.cos(ang).T.astype(np.float32))  # [32, S]
    sinT = np.ascontiguousarray(np.sin(ang).T.astype(np.float32))
    cosq = (np.tile(cosT, (4, 1)) * SCALE).astype(np.float16)
    sinq = (np.concatenate([-sinT, sinT, -sinT, sinT], 0) * SCALE).astype(np.float16)
    cosk = np.tile(cosT, (2, 1)).astype(np.float16)
    sink = np.concatenate([-sinT, sinT], 0).astype(np.float16)

    tri4 = np.tile(np.triu(np.ones((128, 128), np.float16)), (1, 4))

    in_maps = []
    for g in range(N_CORES):
        wqg = w_qkv[g * GCOLS:(g + 1) * GCOLS, :].T          # [HID, 384]
        wq_t = np.ascontiguousarray(
            wqg.reshape(16, 128, GCOLS).transpose(1, 0, 2).reshape(128, 16 * GCOLS)).astype(np.float16)
        bqg = np.ascontiguousarray(
            b_qkv[g * GCOLS:(g + 1) * GCOLS].reshape(3, 128).T)
        wdg = w_dense[:, g * NQ * D:(g + 1) * NQ * D].T      # [256, HID]
        wd_t = np.ascontiguousarray(
            wdg.reshape(2, 128, HID).transpose(1, 0, 2).reshape(128, 2 * HID)).astype(np.float16)
        in_maps.append({
            "ht2": ht2, "wq": wq_t, "bq": bqg, "wd": wd_t,
            "cosq": np.ascontiguousarray(cosq), "sinq": np.ascontiguousarray(sinq),
            "cosk": np.ascontiguousarray(cosk), "sink": np.ascontiguousarray(sink),
            "tri4": tri4,
        })
    return in_maps


def run_device(hidden_states, w_qkv, b_qkv, w_dense, **run_kwargs):
    nc = _get_nc()
    in_maps = _host_inputs(hidden_states, w_qkv, b_qkv, w_dense)
    return run_bass_kernel_spmd(nc, in_maps, list(range(N_CORES)), **run_kwargs)


def kernel(hidden_states, w_qkv, b_qkv, w_dense, b_dense):
    res = run_device(hidden_states, w_qkv, b_qkv, w_dense)
    acc = np.zeros((S, HID), dtype=np.float32)
    for r in res.results:
        acc += r["out"].astype(np.float32)
    acc += np.asarray(b_dense, dtype=np.float32)[None, :]
    return acc.reshape(1, S, HID)


# revision 27
# speedup vs baseline: 1.1232x; 1.0525x over previous
"""TLGv4 block-sparse self-attention on 8 trn2 NeuronCores.

Sharding: tensor-parallel over the 8 KV groups (1 group = 4 Q heads + 1 K +
1 V head per core). Each core computes its group's QKV projection columns,
RoPE, block-sparse attention for its 4 Q heads, and a row-sharded partial of
the dense output projection (written f16). Host sums the 8 partials
(+ b_dense) in f32.

v2 structure - one interleaved stream, per 512-token chunk n:
  - QKV matmuls for chunk n (wq stationary, hidden^T moving, 3 PSUM accs),
    with the largest not-yet-done pair's score matmuls woven between k-steps
    so its Exps (ACT) run under the QKV burst and never pace the PE
  - RoPE on q/k via DVE (q pre-scaled by 1/sqrt(D)); v^T -> v[t,d] via xbar
    DMA transpose; V carries 64 all-ones columns so PV PSUM rows 64:127
    hold the softmax denominators already broadcast across partitions
    (reciprocal + 4 muls, no DRAM bounce)
  - remaining pairs of the group inline: per chunk, score MM -> Exp ->
    sparsity memsets / causal tri multiply, PV lagging 3 chunks, and the
    previous pair's dense-partial units drained between chunks as
    always-ready PE filler
"""
import numpy as np
from contextlib import ExitStack

import concourse.bacc as bacc
import concourse.bass as bass
import concourse.mybir as mybir
import concourse.tile as tile
from concourse.bass_utils import run_bass_kernel_spmd

F32 = mybir.dt.float32
F16 = mybir.dt.float16
AF = mybir.ActivationFunctionType

S = 2048
HID = 2048
D = 64
H_KV = 8
NQ = 4                      # q heads per kv group
GCOLS = (NQ + 2) * D        # 384 qkv columns per group
NPAIR = S // 128            # 16 pairs of 64-token blocks
SCALE = 1.0 / 8.0           # 1/sqrt(D)
ROPE_BASE = 10000.0
N_CORES = 8


def _pair_chunks(i):
    """128-token k-chunks feeding query pair i (blocks 2i, 2i+1)."""
    chunks = list(range(max(0, i - 8), i + 1))
    if i >= 12:
        chunks = [3] + chunks
    return chunks


def _build_nc():
    nc = bacc.Bacc()

    # ht2: host-swizzled so each (n, kq) hidden chunk is one contiguous
    # [128, 1024] read: col = ((n*8 + kq)*2 + c)*512 + t holds
    # h[n*512 + t, kq*256 + c*128 + p]
    ht2 = nc.declare_dram_parameter("ht2", [128, 16 * S], F16, isOutput=False)
    wq = nc.declare_dram_parameter("wq", [128, 16 * GCOLS], F16, isOutput=False)
    bq = nc.declare_dram_parameter("bq", [128, 3], F32, isOutput=False)
    wd = nc.declare_dram_parameter("wd", [128, 2 * HID], F16, isOutput=False)
    cosq = nc.declare_dram_parameter("cosq", [128, S], F16, isOutput=False)
    sinq = nc.declare_dram_parameter("sinq", [128, S], F16, isOutput=False)
    cosk = nc.declare_dram_parameter("cosk", [64, S], F16, isOutput=False)
    sink = nc.declare_dram_parameter("sink", [64, S], F16, isOutput=False)
    tri4 = nc.declare_dram_parameter("tri4", [128, 512], F16, isOutput=False)
    out = nc.declare_dram_parameter("out", [S, HID], F16, isOutput=True)

    scratch = nc.dram_tensor("scratch", [NPAIR, 512], F32)

    with tile.TileContext(nc) as tc, ExitStack() as ctx:
        consts = ctx.enter_context(tc.tile_pool(name="consts", bufs=1))
        persist = ctx.enter_context(tc.tile_pool(name="persist", bufs=1))
        hp = ctx.enter_context(tc.tile_pool(name="hp", bufs=12))
        rp = ctx.enter_context(tc.tile_pool(name="rope", bufs=2))
        att = ctx.enter_context(tc.tile_pool(name="att", bufs=14))
        ob = ctx.enter_context(tc.tile_pool(name="ob", bufs=3))
        small = ctx.enter_context(tc.tile_pool(name="small", bufs=3))
        # PSUM banks: pss 3 + shr (qkv-acc / ctx) 3 + psd 2 = 8
        pss = ctx.enter_context(tc.tile_pool(name="pss", bufs=3, space="PSUM"))
        shr = ctx.enter_context(tc.tile_pool(name="shr", bufs=3, space="PSUM"))
        psd = ctx.enter_context(tc.tile_pool(name="psd", bufs=2, space="PSUM"))

        wq_sb = consts.tile([128, 16 * GCOLS], F16)
        wd_sb = consts.tile([128, 2 * HID], F16)
        bq_sb = consts.tile([128, 3], F32)
        cosq_sb = consts.tile([128, S], F16)
        sinq_sb = consts.tile([128, S], F16)
        cosk_sb = consts.tile([64, S], F16)
        sink_sb = consts.tile([64, S], F16)
        tri_sb = consts.tile([128, 512], F16)
        expb = consts.tile([128, 1], F32)

        # persistent activations
        qkv = [persist.tile([128, S], F16, tag=f"qkv{m}", name=f"qkv{m}")
               for m in range(3)]
        qS = persist.tile([64, NQ * S], F16)      # [d, pair*512 + h*128 + t]
        kT = persist.tile([64, S], F16)           # [d, t]
        v_sb = persist.tile([128, 16 * 128], F16)  # [t, chunk*128 + (d|ones)]
        ctx_sb = persist.tile([128, 2 * S], F16)  # [(h%2)*64+d, (h//2)*2048+t]

        # critical-path-first DMA order: wq chunk 0, first hidden chunks,
        # then small constants
        nc.sync.dma_start(out=wq_sb[:, 0:GCOLS], in_=wq[:, 0:GCOLS])
        hch0 = []
        for kq in range(2):
            hch = hp.tile([128, 1024], F16, tag="hch", name="hch")
            nc.sync.dma_start(out=hch[:],
                              in_=ht2[:, kq * 1024:(kq + 1) * 1024])
            hch0.append(hch)
        nc.vector.memset(expb[:], -5.0)
        for t_, src_ in ((bq_sb, bq), (tri_sb, tri4)):
            nc.sync.dma_start(out=t_[:], in_=src_[:, :])
        v_r = v_sb[:].rearrange("p (c w) -> p c w", w=128)
        nc.vector.memset(v_r[:, :, 64:128], 1.0)

        # ---- dense emission (pair i), split into interleavable units ----
        def dense_units(i):
            ost = ob.tile([128, 2048], F16, tag="ost", name="ost")
            units = []

            def mk(nn):
                def emit():
                    dps = psd.tile([128, 512], F32, tag="d", name="dps")
                    nc.tensor.matmul(dps[:],
                                     ctx_sb[:, i * 128:(i + 1) * 128],
                                     wd_sb[:, nn * 512:(nn + 1) * 512],
                                     start=True, stop=False)
                    nc.tensor.matmul(
                        dps[:],
                        ctx_sb[:, S + i * 128: S + (i + 1) * 128],
                        wd_sb[:, HID + nn * 512: HID + (nn + 1) * 512],
                        start=False, stop=True)
                    if nn % 2 == 0:
                        nc.vector.tensor_copy(ost[:, nn * 512:(nn + 1) * 512],
                                              dps[:])
                    else:
                        nc.scalar.copy(ost[:, nn * 512:(nn + 1) * 512],
                                       dps[:])
                return emit
            for nn in range(4):
                units.append(mk(nn))

            def fin():
                nc.sync.dma_start(out=out[i * 128:(i + 1) * 128, :], in_=ost[:])
            units.append(fin)
            return units

        pend = []   # outstanding dense units
        ready = []  # finished pairs whose dense is not yet queued (lag 1)

        def drain(k):
            for _ in range(min(k, len(pend))):
                pend.pop(0)()

        def pair_done(x):
            # queue pair x's dense one pair later, so its normalize (DMA
            # bounce chain) completes before the dense matmuls drain
            if ready:
                pend.extend(dense_units(ready.pop(0)))
            ready.append(x)

        # ---- attention helpers ----
        def score_chunk(i, c, exs):
            # "half" chunks: only k-rows 64:127 can be visible - compute
            # exp and PV on the upper partition half only (K=64 PV)
            half = (i >= 8 and c == i - 8) or (i >= 12 and c == 3)
            s_ps = pss.tile([128, 512], F32, tag="s", name="s_ps")
            nc.tensor.matmul(s_ps[:], kT[:, c * 128:(c + 1) * 128],
                             qS[:, i * 512:(i + 1) * 512],
                             start=True, stop=True)
            ex = att.tile([128, 512], F16, tag="ex", name="ex")
            if half:
                nc.scalar.activation(ex[64:128, :], s_ps[64:128, :],
                                     AF.Exp, bias=expb[64:128, :])
                if i % 4 != 3 and c == i - 8:
                    exr = ex[64:128, :].rearrange("p (hh t) -> p hh t", hh=NQ)
                    nc.vector.memset(exr[:, :, 64:128], 0.0)
            else:
                nc.scalar.activation(ex[:], s_ps[:], AF.Exp, bias=expb[:])
                if c == i:  # diagonal: causal mask, all heads in one mul
                    nc.vector.tensor_mul(ex[:], ex[:], tri_sb[:])
            exs[c] = (ex, half)

        def pv(ctx_ps, c, exs, start, stop):
            ex, half = exs[c]
            if half:
                nc.tensor.matmul(ctx_ps[:],
                                 v_sb[64:128, c * 128:(c + 1) * 128],
                                 ex[64:128, :], start=start, stop=stop)
            else:
                nc.tensor.matmul(ctx_ps[:], v_sb[:, c * 128:(c + 1) * 128],
                                 ex[:], start=start, stop=stop)

        def finish_pair(i, chunks, exs, done_pv=0):
            """PV tail + softmax normalization for pair i."""
            nch = len(chunks)
            ctx_ps = exs["ctx_ps"]
            for t in range(done_pv, nch):
                pv(ctx_ps, chunks[t], exs, t == 0, t == nch - 1)
            # stage raw ctx + denominator row out of PSUM right away so the
            # shared PSUM rotation never waits on the bounce below
            den = small.tile([1, 512], F32, tag="den", name="den")
            nc.scalar.copy(den[:], ctx_ps[64:65, :])
            ctxr = small.tile([64, 512], F16, tag="ctxr", name="ctxr")
            nc.vector.tensor_copy(ctxr[:], ctx_ps[0:64, :])
            # denominators: DVE `reciprocal` is ~6.5 cyc/elem (3.4us on
            # [64,512]), the fast custom-DVE recip miscomputes on HW here,
            # and ACT Ln/Exp thrashes the activation table. So: [64,8]
            # repack via DMA, 163ns reciprocal, DRAM-bounce partition
            # broadcast (HBM's flat addressing does the cross-partition
            # replication). All three hops ride the idle SWDGE (gpsimd)
            # queues, away from the weight/output streams on HWDGE.
            rec8 = small.tile([64, 8], F32, tag="rec8", name="rec8")
            nc.gpsimd.dma_start(out=rec8[:], in_=den[0:1, :].rearrange(
                "o (p f) -> o p f", p=64))
            nc.vector.reciprocal(rec8[:], rec8[:])
            sc_row = scratch[i:i + 1, :]
            nc.gpsimd.dma_start(out=sc_row.rearrange("o (p f) -> o p f", p=64),
                                in_=rec8[:])
            bcast = small.tile([64, 512], F16, tag="bc", name="bc")
            nc.gpsimd.dma_start(out=bcast[:], in_=bass.AP(
                tensor=sc_row.tensor, offset=sc_row.offset,
                ap=[[0, 64]] + sc_row.ap[1:]))
            for h in range(NQ):
                nc.vector.tensor_mul(
                    ctx_sb[(h % 2) * 64:(h % 2) * 64 + 64,
                           (h // 2) * S + i * 128:(h // 2) * S + (i + 1) * 128],
                    ctxr[:, h * 128:(h + 1) * 128],
                    bcast[:, h * 128:(h + 1) * 128])

        def emit_pair(i):
            """Inline pair: scores with dense drains + lag-3 PV interleave."""
            chunks = _pair_chunks(i)
            nch = len(chunks)
            exs = {}
            exs["ctx_ps"] = shr.tile([128, 512], F32, tag="ps512",
                                     name="ctx_ps")
            ctx_ps = exs["ctx_ps"]
            for t, c in enumerate(chunks):
                score_chunk(i, c, exs)
                if t >= 3:
                    drain(1)
                if t >= 3:
                    pv(ctx_ps, chunks[t - 3], exs, t == 3, False)
            drain(5)
            finish_pair(i, chunks, exs, done_pv=max(0, nch - 3))

        # ---- main interleaved loop over 512-token chunks ----
        for n in range(4):
            nsl = slice(n * 512, (n + 1) * 512)
            # issue this chunk's hidden-state DMAs up front
            hlist = []
            for kq in range(8):
                if n == 0 and kq < 2:
                    hlist.append(hch0[kq])
                    continue
                hch = hp.tile([128, 1024], F16, tag="hch", name="hch")
                nc.sync.dma_start(
                    out=hch[:],
                    in_=ht2[:, (n * 8 + kq) * 1024:(n * 8 + kq + 1) * 1024])
                hlist.append(hch)
                if n == 0:
                    # stagger the remaining qkv weight chunks + dense weights
                    for kn in range(kq * 2 - 3, kq * 2 - 1):
                        if 1 <= kn < 16:
                            nc.sync.dma_start(
                                out=wq_sb[:, kn * GCOLS:(kn + 1) * GCOLS],
                                in_=wq[:, kn * GCOLS:(kn + 1) * GCOLS])
            if n == 0:
                for kn in range(13, 16):
                    nc.sync.dma_start(
                        out=wq_sb[:, kn * GCOLS:(kn + 1) * GCOLS],
                        in_=wq[:, kn * GCOLS:(kn + 1) * GCOLS])

            # delayed pair from the previous group: its scores go between
            # QKV k-steps (exp runs under the QKV burst); two are
            # front-loaded so the PE has ready work across the section
            # boundary
            dchunks, dexs = (None, None)
            if n >= 1:
                dpair = 4 * n - 1
                dchunks = _pair_chunks(dpair)
                dexs = {}
                for c in dchunks[:2]:
                    score_chunk(dpair, c, dexs)

            acc = [shr.tile([128, 512], F32, tag="ps512", name=f"acc{m}")
                   for m in range(3)]
            for kq in range(8):
                if kq >= 4:
                    drain(1)
                hch = hlist[kq]
                for kk in range(2):
                    kc = kq * 2 + kk
                    for mc in range(3):
                        nc.tensor.matmul(
                            acc[mc][:],
                            wq_sb[:, kc * GCOLS + mc * 128: kc * GCOLS + (mc + 1) * 128],
                            hch[:, kk * 512:(kk + 1) * 512],
                            start=(kc == 0), stop=(kc == 15))
                if dchunks is not None and kq + 2 < len(dchunks):
                    score_chunk(dpair, dchunks[kq + 2], dexs)

            for t_, src_ in ((cosq_sb, cosq), (sinq_sb, sinq),
                             (cosk_sb, cosk), (sink_sb, sink)):
                nc.sync.dma_start(out=t_[:, nsl], in_=src_[:, nsl])
            if n == 0:
                # dense weights all in section 0: dense(0) nn units need
                # every wd quarter by the time pair 1 drains them
                for q4 in range(4):
                    nc.sync.dma_start(
                        out=wd_sb[:, q4 * 1024:(q4 + 1) * 1024],
                        in_=wd[:, q4 * 1024:(q4 + 1) * 1024])
            for mc in (2, 0, 1):
                nc.vector.tensor_scalar_add(
                    qkv[mc][:, nsl], acc[mc][:], bq_sb[:, mc:mc + 1])
                if mc == 2:
                    # v^T -> v[t, d] via xbar DMA transpose, issued as soon
                    # as qkv[2] rows 64:128 exist (PVs of this group's pairs
                    # need v_sb)
                    for cc in range(4):
                        c = 4 * n + cc
                        nc.sync.dma_start_transpose(
                            out=v_sb[:, c * 128:c * 128 + 64],
                            in_=qkv[2][64:128, c * 128:(c + 1) * 128])
            # rope on this token chunk
            for ti in range(2):
                qt = qkv[ti]
                rot = rp.tile([128, 512], F16, tag="rot", name="rot")
                for blk in range(4):
                    src = (blk ^ 1) * 32
                    nc.vector.tensor_copy(rot[blk * 32:(blk + 1) * 32, :],
                                          qt[src:src + 32, nsl])
                tmp = rp.tile([128, 512], F16, tag="tmp", name="tmp")
                nc.vector.tensor_mul(tmp[:], qt[:, nsl], cosq_sb[:, nsl])
                nc.vector.tensor_mul(rot[:], rot[:], sinq_sb[:, nsl])
                for half in range(2):  # head 2*ti + half
                    h = 2 * ti + half
                    dst = qS[:, n * 2048:(n + 1) * 2048].rearrange(
                        "p (pp hh t) -> p pp hh t", hh=NQ, t=128)[:, :, h, :]
                    nc.vector.tensor_add(
                        dst,
                        tmp[half * 64:(half + 1) * 64, :].rearrange(
                            "p (pp t) -> p pp t", t=128),
                        rot[half * 64:(half + 1) * 64, :].rearrange(
                            "p (pp t) -> p pp t", t=128))
            # k rope (qkv[2] rows 0:64), reusing rot/tmp slots
            rotk = rp.tile([128, 512], F16, tag="rot", name="rotk")
            nc.vector.tensor_copy(rotk[0:32, :], qkv[2][32:64, nsl])
            nc.vector.tensor_copy(rotk[32:64, :], qkv[2][0:32, nsl])
            tmpk = rp.tile([128, 512], F16, tag="tmp", name="tmpk")
            nc.vector.tensor_mul(tmpk[0:64, :], qkv[2][0:64, nsl],
                                 cosk_sb[:, nsl])
            nc.vector.tensor_mul(rotk[0:64, :], rotk[0:64, :], sink_sb[:, nsl])
            nc.vector.tensor_add(kT[:, nsl], tmpk[0:64, :], rotk[0:64, :])
            # finish the delayed pair, then this group's first three pairs
            if dchunks is not None:
                dexs["ctx_ps"] = shr.tile([128, 512], F32, tag="ps512",
                                          name="ctx_ps")
                drain(5)
                finish_pair(dpair, dchunks, dexs)
                pair_done(dpair)
            for p in range(4 * n, 4 * n + 3):
                emit_pair(p)
                pair_done(p)
        # last pair never got delayed; pair 14's dense drains inside it
        pend.extend(dense_units(ready.pop(0)))
        emit_pair(15)
        pend.extend(dense_units(15))
        drain(len(pend))

    nc.finalize()
    return nc


_NC_CACHE = {}


def _get_nc():
    if "nc" not in _NC_CACHE:
        _NC_CACHE["nc"] = _build_nc()
    return _NC_CACHE["nc"]


def _host_inputs(hidden_states, w_qkv, b_qkv, w_dense):
    h = np.asarray(hidden_states, dtype=np.float32).reshape(S, HID)
    w_qkv = np.asarray(w_qkv, dtype=np.float32)
    b_qkv = np.asarray(b_qkv, dtype=np.float32)
    w_dense = np.asarray(w_dense, dtype=np.float32)

    # contiguous per-(n,kq) [128, 1024] chunks: col ((n*8+kq)*2+c)*512+t
    # holds h[n*512+t, kq*256+c*128+p]
    ht2 = np.ascontiguousarray(
        h.reshape(4, 512, 8, 2, 128).transpose(4, 0, 2, 3, 1).reshape(
            128, 16 * S)).astype(np.float16)

    inv = 1.0 / (ROPE_BASE ** (np.arange(0, D, 2, dtype=np.float32) / D))
    ang = np.arange(S, dtype=np.float32)[:, None] * inv[None, :]   # [S, 32]
    cosT = np.ascontiguousarray(np.cos(ang).T.astype(np.float32))  # [32, S]
    sinT = np.ascontiguousarray(np.sin(ang).T.astype(np.float32))
    cosq = (np.tile(cosT, (4, 1)) * SCALE).astype(np.float16)
    sinq = (np.concatenate([-sinT, sinT, -sinT, sinT], 0) * SCALE).astype(np.float16)
    cosk = np.tile(cosT, (2, 1)).astype(np.float16)
    sink = np.concatenate([-sinT, sinT], 0).astype(np.float16)

    tri4 = np.tile(np.triu(np.ones((128, 128), np.float16)), (1, 4))

    in_maps = []
    for g in range(N_CORES):
        wqg = w_qkv[g * GCOLS:(g + 1) * GCOLS, :].T          # [HID, 384]
        wq_t = np.ascontiguousarray(
            wqg.reshape(16, 128, GCOLS).transpose(1, 0, 2).reshape(128, 16 * GCOLS)).astype(np.float16)
        bqg = np.ascontiguousarray(
            b_qkv[g * GCOLS:(g + 1) * GCOLS].reshape(3, 128).T)
        wdg = w_dense[:, g * NQ * D:(g + 1) * NQ * D].T      # [256, HID]
        wd_t = np.ascontiguousarray(
            wdg.reshape(2, 128, HID).transpose(1, 0, 2).reshape(128, 2 * HID)).astype(np.float16)
        in_maps.append({
            "ht2": ht2, "wq": wq_t, "bq": bqg, "wd": wd_t,
            "cosq": np.ascontiguousarray(cosq), "sinq": np.ascontiguousarray(sinq),
            "cosk": np.ascontiguousarray(cosk), "sink": np.ascontiguousarray(sink),
            "tri4": tri4,
        })
    return in_maps


def run_device(hidden_states, w_qkv, b_qkv, w_dense, **run_kwargs):
    nc = _get_nc()
    in_maps = _host_inputs(hidden_states, w_qkv, b_qkv, w_dense)
    return run_bass_kernel_spmd(nc, in_maps, list(range(N_CORES)), **run_kwargs)


def kernel(hidden_states, w_qkv, b_qkv, w_dense, b_dense):
    res = run_device(hidden_states, w_qkv, b_qkv, w_dense)
    acc = np.zeros((S, HID), dtype=np.float32)
    for r in res.results:
        acc += r["out"].astype(np.float32)
    acc += np.asarray(b_dense, dtype=np.float32)[None, :]
    return acc.reshape(1, S, HID)


# revision 32
# speedup vs baseline: 1.2052x; 1.0730x over previous
"""TLGv4 block-sparse self-attention on 8 trn2 NeuronCores.

Sharding: tensor-parallel over the 8 KV groups (1 group = 4 Q heads + 1 K +
1 V head per core). Each core computes its group's QKV projection columns,
RoPE, block-sparse attention for its 4 Q heads, and a row-sharded partial of
the dense output projection (written f16). Host sums the 8 partials
(+ b_dense) in f32.

v2 structure - one interleaved stream, per 512-token chunk n:
  - QKV matmuls for chunk n (wq stationary, hidden^T moving, 3 PSUM accs),
    with the largest not-yet-done pair's score matmuls woven between k-steps
    so its Exps (ACT) run under the QKV burst and never pace the PE
  - RoPE on q/k via DVE (q pre-scaled by 1/sqrt(D)); v^T -> v[t,d] via xbar
    DMA transpose; V carries 64 all-ones columns so PV PSUM rows 64:127
    hold the softmax denominators already broadcast across partitions
    (reciprocal + 4 muls, no DRAM bounce)
  - remaining pairs of the group inline: per chunk, score MM -> Exp ->
    sparsity memsets / causal tri multiply, PV lagging 3 chunks, and the
    previous pair's dense-partial units drained between chunks as
    always-ready PE filler
"""
import numpy as np
from contextlib import ExitStack

import concourse.bacc as bacc
import concourse.bass as bass
import concourse.mybir as mybir
import concourse.tile as tile
from concourse.bass_utils import run_bass_kernel_spmd

F32 = mybir.dt.float32
F16 = mybir.dt.float16
AF = mybir.ActivationFunctionType

S = 2048
HID = 2048
D = 64
H_KV = 8
NQ = 4                      # q heads per kv group
GCOLS = (NQ + 2) * D        # 384 qkv columns per group
NPAIR = S // 128            # 16 pairs of 64-token blocks
SCALE = 1.0 / 8.0           # 1/sqrt(D)
ROPE_BASE = 10000.0
N_CORES = 8


def _pair_chunks(i):
    """128-token k-chunks feeding query pair i (blocks 2i, 2i+1)."""
    chunks = list(range(max(0, i - 8), i + 1))
    if i >= 12:
        chunks = [3] + chunks
    return chunks


def _build_nc():
    nc = bacc.Bacc()

    # ht2: host-swizzled so each (n, kq) hidden chunk is one contiguous
    # [128, 1024] read: col = ((n*8 + kq)*2 + c)*512 + t holds
    # h[n*512 + t, kq*256 + c*128 + p]
    ht2 = nc.declare_dram_parameter("ht2", [128, 16 * S], F16, isOutput=False)
    wq = nc.declare_dram_parameter("wq", [128, 16 * GCOLS], F16, isOutput=False)
    bq = nc.declare_dram_parameter("bq", [128, 3], F32, isOutput=False)
    wd = nc.declare_dram_parameter("wd", [128, 2 * HID], F16, isOutput=False)
    cosq = nc.declare_dram_parameter("cosq", [128, S], F16, isOutput=False)
    sinq = nc.declare_dram_parameter("sinq", [128, S], F16, isOutput=False)
    cosk = nc.declare_dram_parameter("cosk", [64, S], F16, isOutput=False)
    sink = nc.declare_dram_parameter("sink", [64, S], F16, isOutput=False)
    tri4 = nc.declare_dram_parameter("tri4", [128, 512], F16, isOutput=False)
    out = nc.declare_dram_parameter("out", [S, HID], F16, isOutput=True)

    scratch = nc.dram_tensor("scratch", [NPAIR, 512], F32)

    with tile.TileContext(nc) as tc, ExitStack() as ctx:
        consts = ctx.enter_context(tc.tile_pool(name="consts", bufs=1))
        persist = ctx.enter_context(tc.tile_pool(name="persist", bufs=1))
        hp = ctx.enter_context(tc.tile_pool(name="hp", bufs=12))
        rp = ctx.enter_context(tc.tile_pool(name="rope", bufs=2))
        att = ctx.enter_context(tc.tile_pool(name="att", bufs=14))
        ob = ctx.enter_context(tc.tile_pool(name="ob", bufs=3))
        small = ctx.enter_context(tc.tile_pool(name="small", bufs=3))
        # PSUM banks: pss 3 + shr (qkv-acc / ctx) 3 + psd 2 = 8
        pss = ctx.enter_context(tc.tile_pool(name="pss", bufs=3, space="PSUM"))
        shr = ctx.enter_context(tc.tile_pool(name="shr", bufs=3, space="PSUM"))
        psd = ctx.enter_context(tc.tile_pool(name="psd", bufs=2, space="PSUM"))

        wq_sb = consts.tile([128, 16 * GCOLS], F16)
        wd_sb = consts.tile([128, 2 * HID], F16)
        bq_sb = consts.tile([128, 3], F32)
        cosq_sb = consts.tile([128, S], F16)
        sinq_sb = consts.tile([128, S], F16)
        cosk_sb = consts.tile([64, S], F16)
        sink_sb = consts.tile([64, S], F16)
        tri_sb = consts.tile([128, 512], F16)
        expb = consts.tile([128, 1], F32)

        # persistent activations
        qkv = [persist.tile([128, S], F16, tag=f"qkv{m}", name=f"qkv{m}")
               for m in range(3)]
        qS = persist.tile([64, NQ * S], F16)      # [d, pair*512 + h*128 + t]
        kT = persist.tile([64, S], F16)           # [d, t]
        v_sb = persist.tile([128, 16 * 128], F16)  # [t, chunk*128 + (d|ones)]
        ctx_sb = persist.tile([128, 2 * S], F16)  # [(h%2)*64+d, (h//2)*2048+t]

        # critical-path-first DMA order: wq chunk 0, first hidden chunks,
        # then small constants
        nc.sync.dma_start(out=wq_sb[:, 0:GCOLS], in_=wq[:, 0:GCOLS])
        hch0 = []
        for kq in range(2):
            hch = hp.tile([128, 1024], F16, tag="hch", name="hch")
            nc.sync.dma_start(out=hch[:],
                              in_=ht2[:, kq * 1024:(kq + 1) * 1024])
            hch0.append(hch)
        nc.vector.memset(expb[:], -5.0)
        for t_, src_ in ((bq_sb, bq), (tri_sb, tri4)):
            nc.sync.dma_start(out=t_[:], in_=src_[:, :])
        v_r = v_sb[:].rearrange("p (c w) -> p c w", w=128)
        nc.vector.memset(v_r[:, :, 64:128], 1.0)

        # ---- dense emission (pair i), split into interleavable units ----
        def dense_units(i):
            ost = ob.tile([128, 2048], F16, tag="ost", name="ost")
            units = []

            def mk(nn):
                def emit():
                    dps = psd.tile([128, 512], F32, tag="d", name="dps")
                    nc.tensor.matmul(dps[:],
                                     ctx_sb[:, i * 128:(i + 1) * 128],
                                     wd_sb[:, nn * 512:(nn + 1) * 512],
                                     start=True, stop=False)
                    nc.tensor.matmul(
                        dps[:],
                        ctx_sb[:, S + i * 128: S + (i + 1) * 128],
                        wd_sb[:, HID + nn * 512: HID + (nn + 1) * 512],
                        start=False, stop=True)
                    if nn % 2 == 0:
                        nc.vector.tensor_copy(ost[:, nn * 512:(nn + 1) * 512],
                                              dps[:])
                    else:
                        nc.scalar.copy(ost[:, nn * 512:(nn + 1) * 512],
                                       dps[:])
                return emit
            for nn in range(4):
                units.append(mk(nn))

            def fin():
                nc.sync.dma_start(out=out[i * 128:(i + 1) * 128, :], in_=ost[:])
            units.append(fin)
            return units

        pend = []   # outstanding dense units
        ready = []  # finished pairs whose dense is not yet queued (lag 1)

        def drain(k):
            for _ in range(min(k, len(pend))):
                pend.pop(0)()

        def pair_done(x):
            # queue pair x's dense one pair later, so its normalize (DMA
            # bounce chain) completes before the dense matmuls drain
            if ready:
                pend.extend(dense_units(ready.pop(0)))
            ready.append(x)

        # ---- attention helpers ----
        def score_chunk(i, c, exs):
            # "half" chunks: only k-rows 64:127 can be visible - compute
            # exp and PV on the upper partition half only (K=64 PV)
            half = (i >= 8 and c == i - 8) or (i >= 12 and c == 3)
            s_ps = pss.tile([128, 512], F32, tag="s", name="s_ps")
            nc.tensor.matmul(s_ps[:], kT[:, c * 128:(c + 1) * 128],
                             qS[:, i * 512:(i + 1) * 512],
                             start=True, stop=True)
            ex = att.tile([128, 512], F16, tag="ex", name="ex")
            if half:
                nc.scalar.activation(ex[64:128, :], s_ps[64:128, :],
                                     AF.Exp, bias=expb[64:128, :])
                if i % 4 != 3 and c == i - 8:
                    exr = ex[64:128, :].rearrange("p (hh t) -> p hh t", hh=NQ)
                    nc.vector.memset(exr[:, :, 64:128], 0.0)
            else:
                nc.scalar.activation(ex[:], s_ps[:], AF.Exp, bias=expb[:])
                if c == i:  # diagonal: causal mask, all heads in one mul
                    nc.vector.tensor_mul(ex[:], ex[:], tri_sb[:])
            exs[c] = (ex, half)

        def pv(ctx_ps, c, exs, start, stop):
            ex, half = exs[c]
            if half:
                nc.tensor.matmul(ctx_ps[:],
                                 v_sb[64:128, c * 128:(c + 1) * 128],
                                 ex[64:128, :], start=start, stop=stop)
            else:
                nc.tensor.matmul(ctx_ps[:], v_sb[:, c * 128:(c + 1) * 128],
                                 ex[:], start=start, stop=stop)

        def finish_pair(i, chunks, exs, done_pv=0, last=False):
            """PV tail + softmax normalization for pair i."""
            nch = len(chunks)
            ctx_ps = exs["ctx_ps"]
            for t in range(done_pv, nch):
                pv(ctx_ps, chunks[t], exs, t == 0, t == nch - 1)
            if last:
                # tail: no DMA bounce to wait on - the slow DVE reciprocal
                # (3.4us) on the broadcast rows beats 3 DMA hops when
                # nothing else needs the DVE
                recb = small.tile([64, 512], F32, tag="rec64", name="rec64")
                nc.vector.reciprocal(recb[:], ctx_ps[64:128, :])
                for h in range(NQ):
                    nc.vector.tensor_mul(
                        ctx_sb[(h % 2) * 64:(h % 2) * 64 + 64,
                               (h // 2) * S + i * 128:
                               (h // 2) * S + (i + 1) * 128],
                        ctx_ps[0:64, h * 128:(h + 1) * 128],
                        recb[:, h * 128:(h + 1) * 128])
                return
            # stage raw ctx + denominator row out of PSUM right away so the
            # shared PSUM rotation never waits on the bounce below
            den = small.tile([1, 512], F32, tag="den", name="den")
            nc.scalar.copy(den[:], ctx_ps[64:65, :])
            ctxr = small.tile([64, 512], F16, tag="ctxr", name="ctxr")
            nc.vector.tensor_copy(ctxr[:], ctx_ps[0:64, :])
            # denominators: DVE `reciprocal` is ~6.5 cyc/elem (3.4us on
            # [64,512]), the fast custom-DVE recip miscomputes on HW here,
            # and ACT Ln/Exp thrashes the activation table. So: [64,8]
            # repack via DMA, 163ns reciprocal, DRAM-bounce partition
            # broadcast (HBM's flat addressing does the cross-partition
            # replication). All three hops ride the idle SWDGE (gpsimd)
            # queues, away from the weight/output streams on HWDGE.
            rec8 = small.tile([64, 8], F32, tag="rec8", name="rec8")
            nc.gpsimd.dma_start(out=rec8[:], in_=den[0:1, :].rearrange(
                "o (p f) -> o p f", p=64))
            nc.vector.reciprocal(rec8[:], rec8[:])
            sc_row = scratch[i:i + 1, :]
            nc.gpsimd.dma_start(out=sc_row.rearrange("o (p f) -> o p f", p=64),
                                in_=rec8[:])
            bcast = small.tile([64, 512], F16, tag="bc", name="bc")
            nc.gpsimd.dma_start(out=bcast[:], in_=bass.AP(
                tensor=sc_row.tensor, offset=sc_row.offset,
                ap=[[0, 64]] + sc_row.ap[1:]))
            for h in range(NQ):
                nc.vector.tensor_mul(
                    ctx_sb[(h % 2) * 64:(h % 2) * 64 + 64,
                           (h // 2) * S + i * 128:(h // 2) * S + (i + 1) * 128],
                    ctxr[:, h * 128:(h + 1) * 128],
                    bcast[:, h * 128:(h + 1) * 128])

        def emit_pair(i, last=False):
            """Inline pair: scores with dense drains + lag-3 PV interleave."""
            chunks = _pair_chunks(i)
            nch = len(chunks)
            exs = {}
            exs["ctx_ps"] = shr.tile([128, 512], F32, tag="ps512",
                                     name="ctx_ps")
            ctx_ps = exs["ctx_ps"]
            for t, c in enumerate(chunks):
                score_chunk(i, c, exs)
                if t >= 3:
                    drain(1)
                    pv(ctx_ps, chunks[t - 3], exs, t == 3, False)
            drain(5)
            finish_pair(i, chunks, exs, done_pv=max(0, nch - 3), last=last)

        # ---- main interleaved loop over 512-token chunks ----
        for n in range(4):
            nsl = slice(n * 512, (n + 1) * 512)
            # issue this chunk's hidden-state DMAs up front
            hlist = []
            for kq in range(8):
                if n == 0 and kq < 2:
                    hlist.append(hch0[kq])
                    continue
                hch = hp.tile([128, 1024], F16, tag="hch", name="hch")
                nc.sync.dma_start(
                    out=hch[:],
                    in_=ht2[:, (n * 8 + kq) * 1024:(n * 8 + kq + 1) * 1024])
                hlist.append(hch)
                if n == 0:
                    # stagger the remaining qkv weight chunks + dense weights
                    for kn in range(kq * 2 - 3, kq * 2 - 1):
                        if 1 <= kn < 16:
                            nc.sync.dma_start(
                                out=wq_sb[:, kn * GCOLS:(kn + 1) * GCOLS],
                                in_=wq[:, kn * GCOLS:(kn + 1) * GCOLS])
            if n == 0:
                for kn in range(13, 16):
                    nc.sync.dma_start(
                        out=wq_sb[:, kn * GCOLS:(kn + 1) * GCOLS],
                        in_=wq[:, kn * GCOLS:(kn + 1) * GCOLS])

            # delayed pair from the previous group: its scores go between
            # QKV k-steps (exp runs under the QKV burst); two are
            # front-loaded so the PE has ready work across the section
            # boundary
            dchunks, dexs = (None, None)
            if n >= 1:
                dpair = 4 * n - 1
                dchunks = _pair_chunks(dpair)
                dexs = {}
                for c in dchunks[:2]:
                    score_chunk(dpair, c, dexs)

            acc = [shr.tile([128, 512], F32, tag="ps512", name=f"acc{m}")
                   for m in range(3)]
            for kq in range(8):
                hch = hlist[kq]
                for kk in range(2):
                    kc = kq * 2 + kk
                    for mc in range(3):
                        nc.tensor.matmul(
                            acc[mc][:],
                            wq_sb[:, kc * GCOLS + mc * 128: kc * GCOLS + (mc + 1) * 128],
                            hch[:, kk * 512:(kk + 1) * 512],
                            start=(kc == 0), stop=(kc == 15))
                if dchunks is not None and kq + 2 < len(dchunks):
                    score_chunk(dpair, dchunks[kq + 2], dexs)

            for t_, src_ in ((cosq_sb, cosq), (sinq_sb, sinq),
                             (cosk_sb, cosk), (sink_sb, sink)):
                nc.sync.dma_start(out=t_[:, nsl], in_=src_[:, nsl])
            if n == 0:
                # dense weights all in section 0: dense(0) nn units need
                # every wd quarter by the time pair 1 drains them
                for q4 in range(4):
                    nc.sync.dma_start(
                        out=wd_sb[:, q4 * 1024:(q4 + 1) * 1024],
                        in_=wd[:, q4 * 1024:(q4 + 1) * 1024])
            # bias + rope + v-transpose at 256-token granularity: the first
            # half unblocks pairs 4n/4n+1's scores several us sooner, which
            # shrinks the PE stall in the section-boundary rope window
            for hs in range(2):
                sl = slice(n * 512 + hs * 256, n * 512 + (hs + 1) * 256)
                asl = slice(hs * 256, (hs + 1) * 256)
                for mc in (2, 0, 1):
                    nc.vector.tensor_scalar_add(
                        qkv[mc][:, sl], acc[mc][:, asl], bq_sb[:, mc:mc + 1])
                    if mc == 2:
                        # v^T -> v[t, d] via xbar DMA transpose, as soon as
                        # qkv[2] rows 64:128 exist
                        for cc in range(2 * hs, 2 * hs + 2):
                            c = 4 * n + cc
                            nc.sync.dma_start_transpose(
                                out=v_sb[:, c * 128:c * 128 + 64],
                                in_=qkv[2][64:128, c * 128:(c + 1) * 128])
                # k rope first (scores of the next pairs need kT)
                rotk = rp.tile([128, 512], F16, tag="rot", name="rotk")
                nc.vector.tensor_copy(rotk[0:32, 0:256], qkv[2][32:64, sl])
                nc.vector.tensor_copy(rotk[32:64, 0:256], qkv[2][0:32, sl])
                tmpk = rp.tile([128, 512], F16, tag="tmp", name="tmpk")
                nc.vector.tensor_mul(tmpk[0:64, 0:256], qkv[2][0:64, sl],
                                     cosk_sb[:, sl])
                nc.vector.tensor_mul(rotk[0:64, 0:256], rotk[0:64, 0:256],
                                     sink_sb[:, sl])
                nc.vector.tensor_add(kT[:, sl], tmpk[0:64, 0:256],
                                     rotk[0:64, 0:256])
                for ti in range(2):
                    qt = qkv[ti]
                    rot = rp.tile([128, 512], F16, tag="rot", name="rot")
                    for blk in range(4):
                        src = (blk ^ 1) * 32
                        nc.vector.tensor_copy(rot[blk * 32:(blk + 1) * 32, 0:256],
                                              qt[src:src + 32, sl])
                    tmp = rp.tile([128, 512], F16, tag="tmp", name="tmp")
                    nc.vector.tensor_mul(tmp[:, 0:256], qt[:, sl],
                                         cosq_sb[:, sl])
                    nc.vector.tensor_mul(rot[:, 0:256], rot[:, 0:256],
                                         sinq_sb[:, sl])
                    for half in range(2):  # head 2*ti + half
                        h = 2 * ti + half
                        dst = qS[:, n * 2048 + hs * 1024:
                                 n * 2048 + (hs + 1) * 1024].rearrange(
                            "p (pp hh t) -> p pp hh t", hh=NQ, t=128)[:, :, h, :]
                        nc.vector.tensor_add(
                            dst,
                            tmp[half * 64:(half + 1) * 64, 0:256].rearrange(
                                "p (pp t) -> p pp t", t=128),
                            rot[half * 64:(half + 1) * 64, 0:256].rearrange(
                                "p (pp t) -> p pp t", t=128))
            # finish the delayed pair, then this group's first three pairs
            if dchunks is not None:
                dexs["ctx_ps"] = shr.tile([128, 512], F32, tag="ps512",
                                          name="ctx_ps")
                drain(5)
                finish_pair(dpair, dchunks, dexs)
                pair_done(dpair)
            for p in range(4 * n, 4 * n + 3):
                emit_pair(p)
                pair_done(p)
        # last pair never got delayed; pair 14's dense drains inside it
        pend.extend(dense_units(ready.pop(0)))
        emit_pair(15, last=True)
        pend.extend(dense_units(15))
        drain(len(pend))

    nc.finalize()
    return nc


_NC_CACHE = {}


def _get_nc():
    if "nc" not in _NC_CACHE:
        _NC_CACHE["nc"] = _build_nc()
    return _NC_CACHE["nc"]


def _host_inputs(hidden_states, w_qkv, b_qkv, w_dense):
    h = np.asarray(hidden_states, dtype=np.float32).reshape(S, HID)
    w_qkv = np.asarray(w_qkv, dtype=np.float32)
    b_qkv = np.asarray(b_qkv, dtype=np.float32)
    w_dense = np.asarray(w_dense, dtype=np.float32)

    # contiguous per-(n,kq) [128, 1024] chunks: col ((n*8+kq)*2+c)*512+t
    # holds h[n*512+t, kq*256+c*128+p]
    ht2 = np.ascontiguousarray(
        h.reshape(4, 512, 8, 2, 128).transpose(4, 0, 2, 3, 1).reshape(
            128, 16 * S)).astype(np.float16)

    inv = 1.0 / (ROPE_BASE ** (np.arange(0, D, 2, dtype=np.float32) / D))
    ang = np.arange(S, dtype=np.float32)[:, None] * inv[None, :]   # [S, 32]
    cosT = np.ascontiguousarray(np.cos(ang).T.astype(np.float32))  # [32, S]
    sinT = np.ascontiguousarray(np.sin(ang).T.astype(np.float32))
    cosq = (np.tile(cosT, (4, 1)) * SCALE).astype(np.float16)
    sinq = (np.concatenate([-sinT, sinT, -sinT, sinT], 0) * SCALE).astype(np.float16)
    cosk = np.tile(cosT, (2, 1)).astype(np.float16)
    sink = np.concatenate([-sinT, sinT], 0).astype(np.float16)

    tri4 = np.tile(np.triu(np.ones((128, 128), np.float16)), (1, 4))

    in_maps = []
    for g in range(N_CORES):
        wqg = w_qkv[g * GCOLS:(g + 1) * GCOLS, :].T          # [HID, 384]
        wq_t = np.ascontiguousarray(
            wqg.reshape(16, 128, GCOLS).transpose(1, 0, 2).reshape(128, 16 * GCOLS)).astype(np.float16)
        bqg = np.ascontiguousarray(
            b_qkv[g * GCOLS:(g + 1) * GCOLS].reshape(3, 128).T)
        wdg = w_dense[:, g * NQ * D:(g + 1) * NQ * D].T      # [256, HID]
        wd_t = np.ascontiguousarray(
            wdg.reshape(2, 128, HID).transpose(1, 0, 2).reshape(128, 2 * HID)).astype(np.float16)
        in_maps.append({
            "ht2": ht2, "wq": wq_t, "bq": bqg, "wd": wd_t,
            "cosq": np.ascontiguousarray(cosq), "sinq": np.ascontiguousarray(sinq),
            "cosk": np.ascontiguousarray(cosk), "sink": np.ascontiguousarray(sink),
            "tri4": tri4,
        })
    return in_maps


def run_device(hidden_states, w_qkv, b_qkv, w_dense, **run_kwargs):
    nc = _get_nc()
    in_maps = _host_inputs(hidden_states, w_qkv, b_qkv, w_dense)
    return run_bass_kernel_spmd(nc, in_maps, list(range(N_CORES)), **run_kwargs)


def kernel(hidden_states, w_qkv, b_qkv, w_dense, b_dense):
    res = run_device(hidden_states, w_qkv, b_qkv, w_dense)
    acc = np.zeros((S, HID), dtype=np.float32)
    for r in res.results:
        acc += r["out"].astype(np.float32)
    acc += np.asarray(b_dense, dtype=np.float32)[None, :]
    return acc.reshape(1, S, HID)


# revision 33
# speedup vs baseline: 1.3036x; 1.0816x over previous
"""TLGv4 block-sparse self-attention on 8 trn2 NeuronCores.

Sharding: tensor-parallel over the 8 KV groups (1 group = 4 Q heads + 1 K +
1 V head per core). Each core computes its group's QKV projection columns,
RoPE, block-sparse attention for its 4 Q heads, and a row-sharded partial of
the dense output projection. Host sums the 8 partials (+ b_dense).

Device dataflow per core (all matmuls fp32r = fp32 with 11-bit mantissa,
full PE rate at N=512):
  - qkvT[c, t] = wq_g @ hidden^T     (channels on partitions, tokens free)
  - RoPE on q/k rows via DVE with host-built cos/sin tables (q pre-scaled
    by 1/sqrt(D)); q repacked to qS[d, pair*512 + head*128 + t]
  - v^T transposed back to v[t, d] via PE, augmented with a ones column
  - per 128-query pair: scores^T[k, (h,q)] chunks via matmul, Exp on ACT
    (no max subtraction needed: |scores| < ~10), block-sparsity via memsets
    and a causal 0/1 mask multiply, PV accumulated over chunks; the ones
    column of V yields softmax denominators as ctx row 64; reciprocal is
    partition-broadcast via a DRAM bounce and multiplied in
  - dense partial: out[t, :] = ctx^T_g @ wd_g rows, streamed to DRAM
"""
import numpy as np
from contextlib import ExitStack

import concourse.bacc as bacc
import concourse.bass as bass
import concourse.mybir as mybir
import concourse.tile as tile
from concourse.bass_utils import run_bass_kernel_spmd

F32 = mybir.dt.float32
F32R = mybir.dt.float32r
F16 = mybir.dt.float16
AF = mybir.ActivationFunctionType

S = 2048
HID = 2048
D = 64
H_KV = 8
NQ = 4                      # q heads per kv group
GCOLS = (NQ + 2) * D        # 384 qkv columns per group
NPAIR = S // 128            # 16 pairs of 64-token blocks
SCALE = 1.0 / 8.0           # 1/sqrt(D)
ROPE_BASE = 10000.0
N_CORES = 8


def _r32r(x):
    u = np.ascontiguousarray(x, dtype=np.float32).view(np.uint32)
    u = (u.astype(np.uint64) + 0x800) & 0xFFFFF000
    return u.astype(np.uint32).view(np.float32).reshape(x.shape)


def _pair_chunks(i):
    """128-token k-chunks feeding query pair i (blocks 2i, 2i+1)."""
    chunks = list(range(max(0, i - 8), i + 1))
    if i >= 12:
        chunks = [3] + chunks
    return chunks


def _build_nc():
    nc = bacc.Bacc()

    ht = nc.declare_dram_parameter("ht", [HID, S], F16, isOutput=False)
    wq = nc.declare_dram_parameter("wq", [128, 16 * GCOLS], F16, isOutput=False)
    bq = nc.declare_dram_parameter("bq", [128, 3], F32, isOutput=False)
    wd = nc.declare_dram_parameter("wd", [128, 2 * HID], F16, isOutput=False)
    cosq = nc.declare_dram_parameter("cosq", [128, S], F16, isOutput=False)
    sinq = nc.declare_dram_parameter("sinq", [128, S], F16, isOutput=False)
    cosk = nc.declare_dram_parameter("cosk", [64, S], F16, isOutput=False)
    sink = nc.declare_dram_parameter("sink", [64, S], F16, isOutput=False)
    tri = nc.declare_dram_parameter("tri", [128, 128], F16, isOutput=False)
    ident = nc.declare_dram_parameter("ident", [128, 128], F16, isOutput=False)
    out = nc.declare_dram_parameter("out", [S, HID], F32, isOutput=True)

    scratch = nc.dram_tensor("scratch", [NPAIR, 512], F32)

    with tile.TileContext(nc) as tc, ExitStack() as ctx:
        consts = ctx.enter_context(tc.tile_pool(name="consts", bufs=1))
        persist = ctx.enter_context(tc.tile_pool(name="persist", bufs=1))

        wq_sb = consts.tile([128, 16 * GCOLS], F16)
        wd_sb = consts.tile([128, 2 * HID], F16)
        bq_sb = consts.tile([128, 3], F32)
        cosq_sb = consts.tile([128, S], F16)
        sinq_sb = consts.tile([128, S], F16)
        cosk_sb = consts.tile([64, S], F16)
        sink_sb = consts.tile([64, S], F16)
        tri_sb = consts.tile([128, 128], F16)
        id_sb = consts.tile([128, 128], F16)
        expb = consts.tile([128, 1], F32)
        nc.vector.memset(expb[:], -5.0)
        # small/early constants first; weight chunks split for fast start
        for t_, src in ((bq_sb, bq), (tri_sb, tri), (id_sb, ident)):
            nc.sync.dma_start(out=t_[:], in_=src[:, :])
        nc.sync.dma_start(out=wq_sb[:, 0:GCOLS], in_=wq[:, 0:GCOLS])

        # persistent activations
        qkv = [persist.tile([128, S], F16, tag=f"qkv{m}", name=f"qkv{m}")
               for m in range(3)]
        qS = persist.tile([64, NQ * S], F16)     # [d, pair*512 + h*128 + t]
        kT = persist.tile([64, S], F16)          # [d, t]
        v_sb = persist.tile([128, 16 * 66], F16)  # [t, chunk*66 + (d | ones | pad)]
        ctx_sb = persist.tile([128, 2 * S], F16)  # [(h%2)*64+d, (h//2)*2048 + t]

        v_r = v_sb[:].rearrange("p (c w) -> p c w", w=66)
        nc.vector.memset(v_r[:, :, 65:66], 0.0)
        nc.vector.memset(v_r[:, :, 64:65], 1.0)

        # ---- QKV + RoPE + V-transpose pipeline, per 512-token chunk ----
        with tc.tile_pool(name="hp", bufs=8) as hp, \
             tc.tile_pool(name="rope", bufs=2) as rp, \
             tc.tile_pool(name="psq", bufs=2, space="PSUM") as psq, \
             tc.tile_pool(name="pst", bufs=2, space="PSUM") as pst:
            for n in range(4):
                nsl = slice(n * 512, (n + 1) * 512)
                acc = [psq.tile([128, 512], F32, tag=f"a{m}", name=f"acc{m}")
                       for m in range(3)]
                for kq in range(8):
                    hch = hp.tile([128, 1024], F16)
                    src = ht[kq * 256:(kq + 1) * 256, nsl].rearrange(
                        "(c p) t -> p c t", p=128)
                    nc.sync.dma_start(out=hch[:].rearrange(
                        "p (c t) -> p c t", c=2), in_=src)
                    if n == 0:
                        for kn in range(kq * 2 + 1, kq * 2 + 3):
                            if kn < 16:
                                nc.sync.dma_start(
                                    out=wq_sb[:, kn * GCOLS:(kn + 1) * GCOLS],
                                    in_=wq[:, kn * GCOLS:(kn + 1) * GCOLS])
                    for kk in range(2):
                        kc = kq * 2 + kk
                        for mc in range(3):
                            nc.tensor.matmul(
                                acc[mc][:],
                                wq_sb[:, kc * GCOLS + mc * 128: kc * GCOLS + (mc + 1) * 128],
                                hch[:, kk * 512:(kk + 1) * 512],
                                start=(kc == 0), stop=(kc == 15))
                # later-phase constants ride the SWDGE queues (no contention
                # with the hidden-state stream on HWDGE)
                for t_, src in ((cosq_sb, cosq), (sinq_sb, sinq),
                                (cosk_sb, cosk), (sink_sb, sink)):
                    nc.sync.dma_start(out=t_[:, nsl], in_=src[:, nsl])
                nc.sync.dma_start(out=wd_sb[:, n * 1024:(n + 1) * 1024],
                                  in_=wd[:, n * 1024:(n + 1) * 1024])
                for mc in range(3):
                    nc.vector.tensor_scalar_add(
                        qkv[mc][:, nsl], acc[mc][:], bq_sb[:, mc:mc + 1])
                # rope on this token chunk
                for ti in range(2):
                    qt = qkv[ti]
                    rot = rp.tile([128, 512], F16, tag="rot", name="rot")
                    for blk in range(4):
                        src = (blk ^ 1) * 32
                        nc.vector.tensor_copy(rot[blk * 32:(blk + 1) * 32, :],
                                              qt[src:src + 32, nsl])
                    tmp = rp.tile([128, 512], F16, tag="tmp", name="tmp")
                    nc.vector.tensor_mul(tmp[:], qt[:, nsl],
                                         cosq_sb[:, nsl])
                    nc.vector.tensor_mul(rot[:], rot[:], sinq_sb[:, nsl])
                    for half in range(2):  # head 2*ti + half
                        h = 2 * ti + half
                        dst = qS[:, n * 2048:(n + 1) * 2048].rearrange(
                            "p (pp hh t) -> p pp hh t", hh=NQ, t=128)[:, :, h, :]
                        nc.vector.tensor_add(
                            dst,
                            tmp[half * 64:(half + 1) * 64, :].rearrange(
                                "p (pp t) -> p pp t", t=128),
                            rot[half * 64:(half + 1) * 64, :].rearrange(
                                "p (pp t) -> p pp t", t=128))
                # k rope (qkv[2] rows 0:64), reusing rot/tmp slots
                rotk = rp.tile([128, 512], F16, tag="rot", name="rotk")
                nc.vector.tensor_copy(rotk[0:32, :], qkv[2][32:64, nsl])
                nc.vector.tensor_copy(rotk[32:64, :], qkv[2][0:32, nsl])
                tmpk = rp.tile([128, 512], F16, tag="tmp", name="tmpk")
                nc.vector.tensor_mul(tmpk[0:64, :], qkv[2][0:64, nsl],
                                     cosk_sb[:, nsl])
                nc.vector.tensor_mul(rotk[0:64, :], rotk[0:64, :], sink_sb[:, nsl])
                nc.vector.tensor_add(kT[:, nsl], tmpk[0:64, :], rotk[0:64, :])
                # v transpose for the 4 128-token chunks in this slice
                for cc in range(4):
                    c = 4 * n + cc
                    pt = pst.tile([128, 64], F16, name="pt")
                    nc.tensor.transpose(pt[:],
                                        qkv[2][64:128, c * 128:(c + 1) * 128],
                                        id_sb[64:128, 64:128])
                    nc.vector.tensor_copy(v_sb[:, c * 66:c * 66 + 64], pt[:])

        # ---- attention pairs + deferred dense ----
        # pair 11 first: it only needs rope(<=2) so it hides the rope(3) tail;
        # then big pairs descending; tiny pairs last under the dense backlog
        PAIR_ORDER = list(range(NPAIR))
        with tc.tile_pool(name="pss", bufs=3, space="PSUM") as pss, \
             tc.tile_pool(name="psc", bufs=3, space="PSUM") as psc, \
             tc.tile_pool(name="psd", bufs=2, space="PSUM") as psd, \
             tc.tile_pool(name="att", bufs=12) as att, \
             tc.tile_pool(name="ob", bufs=3) as ob, \
             tc.tile_pool(name="small", bufs=3) as small:

            def emit_dense(i):
                for nn in range(4):
                    dps = psd.tile([128, 512], F32, name="dps")
                    nc.tensor.matmul(dps[:],
                                     ctx_sb[:, i * 128:(i + 1) * 128],
                                     wd_sb[:, nn * 512:(nn + 1) * 512],
                                     start=True, stop=False)
                    nc.tensor.matmul(dps[:],
                                     ctx_sb[:, S + i * 128: S + (i + 1) * 128],
                                     wd_sb[:, HID + nn * 512: HID + (nn + 1) * 512],
                                     start=False, stop=True)
                    ost = ob.tile([128, 512], F32, name="ost")
                    if nn % 2 == 0:
                        nc.vector.tensor_copy(ost[:], dps[:])
                    else:
                        nc.scalar.copy(ost[:], dps[:])
                    nc.sync.dma_start(
                        out=out[i * 128:(i + 1) * 128, nn * 512:(nn + 1) * 512],
                        in_=ost[:])

            def emit_pair(i):
                chunks = _pair_chunks(i)
                ctx_ps = psc.tile([66, 512], F32, name="ctx_ps")
                exs = []
                # all score matmuls first: exp/masks complete in their shadow,
                # so the PV matmuls below never wait on ACT/DVE
                for c in chunks:
                    s_ps = pss.tile([128, 512], F32, name="s_ps")
                    nc.tensor.matmul(s_ps[:], kT[:, c * 128:(c + 1) * 128],
                                     qS[:, i * 512:(i + 1) * 512],
                                     start=True, stop=True)
                    ex = att.tile([128, 512], F16, tag="ex", name="ex")
                    nc.scalar.activation(ex[:], s_ps[:], AF.Exp, bias=expb[:])
                    exs.append(ex)
                    if c == i:  # diagonal: causal mask per head
                        for h in range(NQ):
                            nc.vector.tensor_mul(ex[:, h * 128:(h + 1) * 128],
                                                 ex[:, h * 128:(h + 1) * 128],
                                                 tri_sb[:])
                    elif i >= 8 and c == i - 8:
                        # first half-block invisible; second half only visible
                        # to the odd query block if it is a vertical block
                        nc.vector.memset(ex[0:64, :], 0.0)
                        if i % 4 != 3:
                            exr = ex[64:128, :].rearrange(
                                "p (hh t) -> p hh t", hh=NQ)
                            nc.vector.memset(exr[:, :, 64:128], 0.0)
                    elif i >= 12 and c == 3:
                        # vertical block 7 lives in chunk 3; block 6 invisible
                        nc.vector.memset(ex[0:64, :], 0.0)
                for idx, c in enumerate(chunks):
                    nc.tensor.matmul(ctx_ps[:], v_sb[:, c * 66:(c + 1) * 66],
                                     exs[idx][:], start=(idx == 0),
                                     stop=(idx == len(chunks) - 1))
                # denominators: scatter to [64,8], fast reciprocal, DRAM
                # bounce, partition-broadcast back as [64,512]
                den = small.tile([1, 512], F32, tag="den", name="den")
                nc.scalar.copy(den[:], ctx_ps[64:65, :])
                rec8 = small.tile([64, 8], F32, tag="rec8", name="rec8")
                nc.sync.dma_start(out=rec8[:], in_=den[0:1, :].rearrange(
                    "o (p f) -> o p f", p=64))
                nc.vector.reciprocal(rec8[:], rec8[:])
                sc_row = scratch[i:i + 1, :]
                nc.sync.dma_start(out=sc_row.rearrange("o (p f) -> o p f", p=64),
                                  in_=rec8[:])
                bcast = small.tile([64, 512], F32, tag="bc", name="bc")
                nc.sync.dma_start(out=bcast[:], in_=bass.AP(
                    tensor=sc_row.tensor, offset=sc_row.offset,
                    ap=[[0, 64]] + sc_row.ap[1:]))
                for h in range(NQ):
                    nc.vector.tensor_mul(
                        ctx_sb[(h % 2) * 64:(h % 2) * 64 + 64,
                               (h // 2) * S + i * 128:(h // 2) * S + (i + 1) * 128],
                        ctx_ps[0:64, h * 128:(h + 1) * 128],
                        bcast[:, h * 128:(h + 1) * 128])

            for pidx, i in enumerate(PAIR_ORDER):
                emit_pair(i)
                if pidx >= 3:
                    emit_dense(PAIR_ORDER[pidx - 3])
            for i in PAIR_ORDER[-3:]:
                emit_dense(i)

    nc.finalize()
    return nc


_NC_CACHE = {}


def _get_nc():
    if "nc" not in _NC_CACHE:
        _NC_CACHE["nc"] = _build_nc()
    return _NC_CACHE["nc"]


def _host_inputs(hidden_states, w_qkv, b_qkv, w_dense):
    h = np.asarray(hidden_states, dtype=np.float32).reshape(S, HID)
    w_qkv = np.asarray(w_qkv, dtype=np.float32)
    b_qkv = np.asarray(b_qkv, dtype=np.float32)
    w_dense = np.asarray(w_dense, dtype=np.float32)

    ht = np.ascontiguousarray(h.T).astype(np.float16)

    inv = 1.0 / (ROPE_BASE ** (np.arange(0, D, 2, dtype=np.float32) / D))
    ang = np.arange(S, dtype=np.float32)[:, None] * inv[None, :]   # [S, 32]
    cosT = np.ascontiguousarray(np.cos(ang).T.astype(np.float32))  # [32, S]
    sinT = np.ascontiguousarray(np.sin(ang).T.astype(np.float32))
    cosq = (np.tile(cosT, (4, 1)) * SCALE).astype(np.float16)
    sinq = (np.concatenate([-sinT, sinT, -sinT, sinT], 0) * SCALE).astype(np.float16)
    cosk = np.tile(cosT, (2, 1)).astype(np.float16)
    sink = np.concatenate([-sinT, sinT], 0).astype(np.float16)

    tri = np.triu(np.ones((128, 128), np.float16))
    ident = np.eye(128, dtype=np.float16)

    in_maps = []
    for g in range(N_CORES):
        wqg = w_qkv[g * GCOLS:(g + 1) * GCOLS, :].T          # [HID, 384]
        wq_t = np.ascontiguousarray(
            wqg.reshape(16, 128, GCOLS).transpose(1, 0, 2).reshape(128, 16 * GCOLS)).astype(np.float16)
        bqg = np.ascontiguousarray(
            b_qkv[g * GCOLS:(g + 1) * GCOLS].reshape(3, 128).T)
        wdg = w_dense[:, g * NQ * D:(g + 1) * NQ * D].T      # [256, HID]
        wd_t = np.ascontiguousarray(
            wdg.reshape(2, 128, HID).transpose(1, 0, 2).reshape(128, 2 * HID)).astype(np.float16)
        in_maps.append({
            "ht": ht, "wq": wq_t, "bq": bqg, "wd": wd_t,
            "cosq": np.ascontiguousarray(cosq), "sinq": np.ascontiguousarray(sinq),
            "cosk": np.ascontiguousarray(cosk), "sink": np.ascontiguousarray(sink),
            "tri": tri, "ident": ident,
        })
    return in_maps


def run_device(hidden_states, w_qkv, b_qkv, w_dense, **run_kwargs):
    nc = _get_nc()
    in_maps = _host_inputs(hidden_states, w_qkv, b_qkv, w_dense)
    return run_bass_kernel_spmd(nc, in_maps, list(range(N_CORES)), **run_kwargs)


def kernel(hidden_states, w_qkv, b_qkv, w_dense, b_dense):
    res = run_device(hidden_states, w_qkv, b_qkv, w_dense)
    acc = np.zeros((S, HID), dtype=np.float32)
    for r in res.results:
        acc += r["out"]
    acc += np.asarray(b_dense, dtype=np.float32)[None, :]
    return acc.reshape(1, S, HID)

